# revision 6
# baseline (speedup 1.0000x reference)
"""Multi-head attention (B=4, S=2048, D=1024, H=16) on 8 trn2 NeuronCores.

Sharding: (batch, head-group) -> 8 shards of (1 batch x 8 heads). Zero
cross-core communication: each core computes Q/K/V projections for its 8
heads, full attention over S=2048, and a partial output projection
(row-split Wo); the host sums the two head-group partials per batch.

Layout strategy: the host feeds per-batch inputs pre-transposed ([D, S]) so
every matmul's contraction dim lands on SBUF partitions without any
on-device transposes. The kernel keeps everything in "feature-major" form:
  Q^T, K^T: [dh, s]  -> scores^T[sk, q] = K^T.T-contract  (softmax along
  partitions is avoided via an appended ones-column on V, which makes the
  PV matmul emit the softmax denominator as an extra output row)
  V: natural [s, dh] (+ ones col) -> ctx^T[dh(+1), q]
  out^T[do, q] = Wo_slice^T-contract(ctx^T / rowsum)
All matmuls run as float32r (FP22 truncation, full PE rate).
"""

import numpy as np

import concourse.bass as bass
import concourse.tile as tile
from concourse import bacc, mybir
from concourse.bass_utils import run_bass_kernel_spmd

F32 = mybir.dt.float32
F32R = mybir.dt.float32r
AF = mybir.ActivationFunctionType

B, S, D = 4, 2048, 1024
HPC = 8          # heads per core
DHT = 512        # head dims per core (8 * 64)
NDT = D // 128   # 8 d-tiles (contraction tiles for projections)
NHT = DHT // 128  # 4 dh-tiles
NST = S // 128   # 16 s-tiles
NSB = S // 512   # 4 s-blocks
N_CORES = 8


def _pbc(ap, n):
    """Partition-broadcast AP (step-0 partition dim) for DMA replication."""
    return ap.partition_broadcast(n)


def build_nc():
    nc = bacc.Bacc(None, target_bir_lowering=False)

    xq = nc.declare_dram_parameter("xq_t", [D, S], F32, isOutput=False)
    xk = nc.declare_dram_parameter("xk_t", [D, S], F32, isOutput=False)
    xv = nc.declare_dram_parameter("xv_t", [D, S], F32, isOutput=False)
    wq = nc.declare_dram_parameter("wq", [D, DHT], F32, isOutput=False)
    wk = nc.declare_dram_parameter("wk", [D, DHT], F32, isOutput=False)
    wv = nc.declare_dram_parameter("wv", [D, DHT], F32, isOutput=False)
    wo = nc.declare_dram_parameter("wo", [DHT, D], F32, isOutput=False)
    bq = nc.declare_dram_parameter("bq", [DHT], F32, isOutput=False)
    bk = nc.declare_dram_parameter("bk", [DHT], F32, isOutput=False)
    bv = nc.declare_dram_parameter("bv", [DHT], F32, isOutput=False)
    ot = nc.declare_dram_parameter("o_t", [D, S], F32, isOutput=True)

    # DRAM views tiled to 128 partitions
    xq_v = xq.rearrange("(t p) s -> p t s", p=128)
    xk_v = xk.rearrange("(t p) s -> p t s", p=128)
    xv_v = xv.rearrange("(t p) s -> p t s", p=128)
    wq_v = wq.rearrange("(t p) n -> p t n", p=128)
    wk_v = wk.rearrange("(t p) n -> p t n", p=128)
    wv_v = wv.rearrange("(t p) n -> p t n", p=128)
    wo_v = wo.rearrange("(t p) n -> p t n", p=128)
    ot_v = ot.rearrange("(t p) s -> t p s", p=128)

    with tile.TileContext(nc) as tc:
        with (
            tc.tile_pool(name="persist", bufs=1) as persist,
            tc.tile_pool(name="outp", bufs=4) as outp,
            tc.tile_pool(name="small", bufs=4) as small,
            tc.tile_pool(name="ps_big", bufs=2, space="PSUM") as ps_big,
            tc.tile_pool(name="ps_ctx", bufs=1, space="PSUM") as ps_ctx,
            tc.tile_pool(name="ps_o", bufs=2, space="PSUM") as ps_o,
        ):
            KT = persist.tile([128, NHT, S], F32R)     # K^T  [dh, s]
            QT = persist.tile([128, NHT, S], F32R)     # Q^T  [dh, s]
            Vt = persist.tile([128, NST, HPC, 65], F32R)  # V natural + ones col
            bq_sb = persist.tile([128, NHT], F32)
            bk_sb = persist.tile([128, NHT], F32)
            bv_bc = persist.tile([128, HPC, 64], F32)

            nc.sync.dma_start(out=bq_sb, in_=bq.rearrange("(t p) -> p t", p=128))
            nc.sync.dma_start(out=bk_sb, in_=bk.rearrange("(t p) -> p t", p=128))
            nc.sync.dma_start(
                out=bv_bc, in_=_pbc(bv.rearrange("(h d) -> h d", d=64), 128)
            )
            # ones column for the PV matmul's softmax-denominator row
            # (memset can't emit f32r directly; 1.0f is exact in FP22)
            nc.vector.memset(Vt[:, :, :, 64:65].bitcast(F32), 1.0)

            # ---------------- projections ----------------
            with (
                tc.tile_pool(name="w3", bufs=1) as w3,
                tc.tile_pool(name="xs", bufs=2) as xs,
            ):
                wq_sb = w3.tile([128, NDT, DHT], F32R)
                wk_sb = w3.tile([128, NDT, DHT], F32R)
                wv_sb = w3.tile([128, NDT, DHT], F32R)
                nc.sync.dma_start(out=wq_sb, in_=wq_v.bitcast(F32R))
                nc.sync.dma_start(out=wk_sb, in_=wk_v.bitcast(F32R))
                nc.sync.dma_start(out=wv_sb, in_=wv_v.bitcast(F32R))

                # K^T and Q^T: out[dh-tile, s-block] = sum_d W[d, dh] X^T[d, s]
                for xv_ap, w_sb, dst, b_sb in (
                    (xk_v, wk_sb, KT, bk_sb),
                    (xv_v, wv_sb, None, None),   # V handled below (natural layout)
                    (xq_v, wq_sb, QT, bq_sb),
                ):
                    for sb in range(NSB):
                        ssl = slice(sb * 512, (sb + 1) * 512)
                        xst = xs.tile([128, NDT, 512], F32R, tag="xs")
                        nc.sync.dma_start(out=xst, in_=xv_ap[:, :, ssl].bitcast(F32R))
                        if dst is None:
                            # V projection, natural layout: the X^T tile is
                            # stationary so out[s-tile, dh] has s on partitions
                            for su in range(4):
                                pso = ps_o.tile([128, 512], F32, tag="po")
                                for dt in range(NDT):
                                    nc.tensor.matmul(
                                        pso[:, :],
                                        xst[:, dt, bass.ts(su, 128)],
                                        wv_sb[:, dt, :],
                                        start=(dt == 0),
                                        stop=(dt == NDT - 1),
                                    )
                                stile = sb * 4 + su
                                nc.vector.tensor_add(
                                    out=Vt[:, stile, :, 0:64],
                                    in0=pso.rearrange("p (h d) -> p h d", d=64),
                                    in1=bv_bc,
                                )
                        else:
                            for hp in range(2):
                                ps = ps_big.tile([128, 1024], F32, tag="big")
                                for j in range(2):
                                    ht = 2 * hp + j
                                    for dt in range(NDT):
                                        nc.tensor.matmul(
                                            ps[:, bass.ts(j, 512)],
                                            w_sb[:, dt, bass.ts(ht, 128)],
                                            xst[:, dt, :],
                                            start=(dt == 0),
                                            stop=(dt == NDT - 1),
                                        )
                                for j in range(2):
                                    ht = 2 * hp + j
                                    nc.scalar.activation(
                                        out=dst[:, ht, ssl],
                                        in_=ps[:, bass.ts(j, 512)],
                                        func=AF.Identity,
                                        bias=b_sb[:, ht : ht + 1],
                                    )

            # ---------------- attention + output projection ----------------
            with (
                tc.tile_pool(name="wo_p", bufs=1) as wo_p,
                tc.tile_pool(name="ctxn_p", bufs=1) as ctxn_p,
                tc.tile_pool(name="pexp_p", bufs=3) as pexp_p,
            ):
                wo_sb = wo_p.tile([128, NHT, D], F32R)
                nc.sync.dma_start(out=wo_sb, in_=wo_v.bitcast(F32R))
                ctxn = ctxn_p.tile([128, NHT, S], F32R)

                for qp in range(2):  # q column pairs of 1024
                    q0 = qp * 1024
                    for h in range(HPC):
                        po = 64 * (h % 2)
                        ht = h // 2
                        ctx = ps_ctx.tile([65, 1024], F32, tag="ctx")
                        for sk in range(NST):
                            st = ps_big.tile([128, 1024], F32, tag="big")
                            for j in range(2):
                                nc.tensor.matmul(
                                    st[:, bass.ts(j, 512)],
                                    KT[po : po + 64, ht, bass.ts(sk, 128)],
                                    QT[po : po + 64, ht, q0 + j * 512 : q0 + (j + 1) * 512],
                                    start=True,
                                    stop=True,
                                )
                            pexp = pexp_p.tile([128, 1024], F32R, tag="pexp")
                            nc.scalar.activation(
                                out=pexp, in_=st, func=AF.Exp, scale=0.125
                            )
                            for j in range(2):
                                nc.tensor.matmul(
                                    ctx[:, bass.ts(j, 512)],
                                    Vt[:, sk, h, :],
                                    pexp[:, bass.ts(j, 512)],
                                    start=(sk == 0),
                                    stop=(sk == NST - 1),
                                )
                        for j in range(2):
                            rinv = small.tile([1, 512], F32, tag="rinv")
                            nc.vector.reciprocal(
                                out=rinv, in_=ctx[64:65, bass.ts(j, 512)]
                            )
                            rbc = small.tile([64, 512], F32, tag="rbc")
                            nc.gpsimd.partition_broadcast(rbc, rinv)
                            nc.vector.tensor_mul(
                                out=ctxn[po : po + 64, ht, q0 + j * 512 : q0 + (j + 1) * 512],
                                in0=ctx[0:64, bass.ts(j, 512)],
                                in1=rbc,
                            )
                    # output projection for this q-pair
                    for dot in range(8):
                        for j in range(2):
                            pso = ps_o.tile([128, 512], F32, tag="po")
                            for kt in range(NHT):
                                nc.tensor.matmul(
                                    pso[:, :],
                                    wo_sb[:, kt, bass.ts(dot, 128)],
                                    ctxn[:, kt, q0 + j * 512 : q0 + (j + 1) * 512],
                                    start=(kt == 0),
                                    stop=(kt == NHT - 1),
                                )
                            osb = outp.tile([128, 512], F32, tag="osb")
                            nc.vector.tensor_copy(out=osb, in_=pso)
                            nc.sync.dma_start(
                                out=ot_v[dot, :, q0 + j * 512 : q0 + (j + 1) * 512],
                                in_=osb,
                            )

    nc.compile()
    return nc


_NC_CACHE = None


def _get_nc():
    global _NC_CACHE
    if _NC_CACHE is None:
        _NC_CACHE = build_nc()
    return _NC_CACHE


def kernel(q, k, v, Wq, bq, Wk, bk, Wv, bv, Wo, bo):
    q = np.asarray(q, np.float32)
    k = np.asarray(k, np.float32)
    v = np.asarray(v, np.float32)
    Wq = np.asarray(Wq, np.float32)
    Wk = np.asarray(Wk, np.float32)
    Wv = np.asarray(Wv, np.float32)
    Wo = np.asarray(Wo, np.float32)
    bq = np.asarray(bq, np.float32)
    bk = np.asarray(bk, np.float32)
    bv = np.asarray(bv, np.float32)
    bo = np.asarray(bo, np.float32)

    nc = _get_nc()
    in_maps = []
    for core in range(N_CORES):
        b, hg = core // 2, core % 2
        csl = slice(hg * DHT, (hg + 1) * DHT)
        in_maps.append(
            {
                "xq_t": np.ascontiguousarray(q[b].T),
                "xk_t": np.ascontiguousarray(k[b].T),
                "xv_t": np.ascontiguousarray(v[b].T),
                "wq": np.ascontiguousarray(Wq[:, csl]),
                "wk": np.ascontiguousarray(Wk[:, csl]),
                "wv": np.ascontiguousarray(Wv[:, csl]),
                "wo": np.ascontiguousarray(Wo[csl, :]),
                "bq": np.ascontiguousarray(bq[csl]),
                "bk": np.ascontiguousarray(bk[csl]),
                "bv": np.ascontiguousarray(bv[csl]),
            }
        )
    res = run_bass_kernel_spmd(nc, in_maps, list(range(N_CORES)))
    out = np.empty((B, S, D), np.float32)
    for b in range(B):
        o_t = res.results[2 * b]["o_t"] + res.results[2 * b + 1]["o_t"]
        out[b] = o_t.T + bo
    return out


# revision 11
# speedup vs baseline: 1.3480x; 1.3480x over previous
"""Multi-head attention (B=4, S=2048, D=1024, H=16) on 8 trn2 NeuronCores.

Sharding: (batch, head-group) -> 8 shards of (1 batch x 8 heads). Zero
cross-core communication: each core computes Q/K/V projections for its 8
heads, full attention over S=2048, and a partial output projection
(row-split Wo); the host sums the two head-group partials per batch.

Layout strategy: the host feeds per-batch inputs pre-transposed ([D, S]) so
every matmul's contraction dim lands on SBUF partitions without any
on-device transposes. The kernel keeps everything in "feature-major" form:
  Q^T, K^T: [dh, s]  -> scores^T[sk, q] = K^T.T-contract  (softmax along
  partitions is avoided via an appended ones-column on V, which makes the
  PV matmul emit the softmax denominator as an extra output row)
  V: natural [s, dh] (+ ones col) -> ctx^T[dh(+1), q]
  out^T[do, q] = Wo_slice^T-contract(ctx^T / rowsum)
All matmuls run as float32r (FP22 truncation, full PE rate).
"""

import numpy as np

import concourse.bass as bass
import concourse.tile as tile
from concourse import bacc, mybir
from concourse.bass_utils import run_bass_kernel_spmd

F32 = mybir.dt.float32
F32R = mybir.dt.float32r
AF = mybir.ActivationFunctionType

B, S, D = 4, 2048, 1024
HPC = 8          # heads per core
DHT = 512        # head dims per core (8 * 64)
NDT = D // 128   # 8 d-tiles (contraction tiles for projections)
NHT = DHT // 128  # 4 dh-tiles
NST = S // 128   # 16 s-tiles
NSB = S // 512   # 4 s-blocks
N_CORES = 8


def _pbc(ap, n):
    """Partition-broadcast AP (step-0 partition dim) for DMA replication."""
    return ap.partition_broadcast(n)


def build_nc():
    nc = bacc.Bacc(None, target_bir_lowering=False)

    xq = nc.declare_dram_parameter("xq_t", [D, S], F32, isOutput=False)
    xk = nc.declare_dram_parameter("xk_t", [D, S], F32, isOutput=False)
    xv = nc.declare_dram_parameter("xv_t", [D, S], F32, isOutput=False)
    wq = nc.declare_dram_parameter("wq", [D, DHT], F32, isOutput=False)
    wk = nc.declare_dram_parameter("wk", [D, DHT], F32, isOutput=False)
    wv = nc.declare_dram_parameter("wv", [D, DHT], F32, isOutput=False)
    wo = nc.declare_dram_parameter("wo", [DHT, D], F32, isOutput=False)
    bq = nc.declare_dram_parameter("bq", [DHT], F32, isOutput=False)
    bk = nc.declare_dram_parameter("bk", [DHT], F32, isOutput=False)
    bv = nc.declare_dram_parameter("bv", [DHT], F32, isOutput=False)
    ot = nc.declare_dram_parameter("o_t", [D, S], F32, isOutput=True)

    # DRAM views tiled to 128 partitions
    xq_v = xq.rearrange("(t p) s -> p t s", p=128)
    xk_v = xk.rearrange("(t p) s -> p t s", p=128)
    xv_v = xv.rearrange("(t p) s -> p t s", p=128)
    wq_v = wq.rearrange("(t p) n -> p t n", p=128)
    wk_v = wk.rearrange("(t p) n -> p t n", p=128)
    wv_v = wv.rearrange("(t p) n -> p t n", p=128)
    wo_v = wo.rearrange("(t p) n -> p t n", p=128)
    ot_v = ot.rearrange("(t p) s -> t p s", p=128)

    with tile.TileContext(nc) as tc:
        with (
            tc.tile_pool(name="persist", bufs=1) as persist,
            tc.tile_pool(name="outp", bufs=4) as outp,
            tc.tile_pool(name="ps_big", bufs=2, space="PSUM") as ps_big,
            tc.tile_pool(name="ps_ctx", bufs=1, space="PSUM") as ps_ctx,
            tc.tile_pool(name="ps_o", bufs=2, space="PSUM") as ps_o,
        ):
            KT = persist.tile([128, NHT, S], F32R)     # K^T  [dh, s]
            QT = persist.tile([128, NHT, S], F32R)     # Q^T  [dh, s]
            Vt = persist.tile([128, NST, HPC, 65], F32R)  # V natural + ones col
            bq_sb = persist.tile([128, NHT], F32)
            bk_sb = persist.tile([128, NHT], F32)
            bv_bc = persist.tile([128, HPC, 64], F32)

            nc.sync.dma_start(out=bq_sb, in_=bq.rearrange("(t p) -> p t", p=128))
            nc.sync.dma_start(out=bk_sb, in_=bk.rearrange("(t p) -> p t", p=128))
            nc.sync.dma_start(
                out=bv_bc, in_=_pbc(bv.rearrange("(h d) -> h d", d=64), 128)
            )
            # ones column for the PV matmul's softmax-denominator row
            # (memset can't emit f32r directly; 1.0f is exact in FP22)
            nc.vector.memset(Vt[:, :, :, 64:65].bitcast(F32), 1.0)

            # ---------------- projections ----------------
            with (
                tc.tile_pool(name="w3", bufs=1) as w3,
                tc.tile_pool(name="xs", bufs=2) as xs,
            ):
                wq_sb = w3.tile([128, NDT, DHT], F32R)
                wk_sb = w3.tile([128, NDT, DHT], F32R)
                wv_sb = w3.tile([128, NDT, DHT], F32R)
                nc.sync.dma_start(out=wq_sb, in_=wq_v.bitcast(F32R))
                nc.sync.dma_start(out=wk_sb, in_=wk_v.bitcast(F32R))
                nc.sync.dma_start(out=wv_sb, in_=wv_v.bitcast(F32R))

                # K^T and Q^T: out[dh-tile, s-block] = sum_d W[d, dh] X^T[d, s]
                for xv_ap, w_sb, dst, b_sb in (
                    (xk_v, wk_sb, KT, bk_sb),
                    (xv_v, wv_sb, None, None),   # V handled below (natural layout)
                    (xq_v, wq_sb, QT, bq_sb),
                ):
                    for sb in range(NSB):
                        ssl = slice(sb * 512, (sb + 1) * 512)
                        xst = xs.tile([128, NDT, 512], F32R, tag="xs")
                        nc.sync.dma_start(out=xst, in_=xv_ap[:, :, ssl].bitcast(F32R))
                        if dst is None:
                            # V projection, natural layout: the X^T tile is
                            # stationary so out[s-tile, dh] has s on partitions
                            for su in range(4):
                                pso = ps_o.tile([128, 512], F32, tag="po")
                                for dt in range(NDT):
                                    nc.tensor.matmul(
                                        pso[:, :],
                                        xst[:, dt, bass.ts(su, 128)],
                                        wv_sb[:, dt, :],
                                        start=(dt == 0),
                                        stop=(dt == NDT - 1),
                                    )
                                stile = sb * 4 + su
                                nc.vector.tensor_add(
                                    out=Vt[:, stile, :, 0:64],
                                    in0=pso.rearrange("p (h d) -> p h d", d=64),
                                    in1=bv_bc,
                                )
                        else:
                            for hp in range(2):
                                ps = ps_big.tile([128, 1024], F32, tag="big")
                                for j in range(2):
                                    ht = 2 * hp + j
                                    for dt in range(NDT):
                                        nc.tensor.matmul(
                                            ps[:, bass.ts(j, 512)],
                                            w_sb[:, dt, bass.ts(ht, 128)],
                                            xst[:, dt, :],
                                            start=(dt == 0),
                                            stop=(dt == NDT - 1),
                                        )
                                for j in range(2):
                                    ht = 2 * hp + j
                                    # DVE (not ACT) so the scalar engine
                                    # stays free for the softmax exps
                                    nc.vector.tensor_scalar_add(
                                        out=dst[:, ht, ssl],
                                        in0=ps[:, bass.ts(j, 512)],
                                        scalar1=b_sb[:, ht : ht + 1],
                                    )

            # ---------------- attention + output projection ----------------
            with (
                tc.tile_pool(name="wo_p", bufs=1) as wo_p,
                tc.tile_pool(name="ctxn_p", bufs=1) as ctxn_p,
                tc.tile_pool(name="pexp_p", bufs=2) as pexp_p,
                tc.tile_pool(name="small", bufs=2) as small,
            ):
                wo_sb = wo_p.tile([128, NHT, D], F32R)
                nc.sync.dma_start(out=wo_sb, in_=wo_v.bitcast(F32R))
                ctxn = ctxn_p.tile([128, NHT, S], F32R)

                for qp in range(2):  # q column pairs of 1024
                    q0 = qp * 1024
                    for h in range(HPC):
                        po = 64 * (h % 2)
                        ht = h // 2
                        ctx = ps_ctx.tile([65, 1024], F32, tag="ctx")
                        for sk in range(NST):
                            st = ps_big.tile([128, 1024], F32, tag="big")
                            for j in range(2):
                                nc.tensor.matmul(
                                    st[:, bass.ts(j, 512)],
                                    KT[po : po + 64, ht, bass.ts(sk, 128)],
                                    QT[po : po + 64, ht, q0 + j * 512 : q0 + (j + 1) * 512],
                                    start=True,
                                    stop=True,
                                )
                            pexp = pexp_p.tile([128, 1024], F32R, tag="pexp")
                            nc.scalar.activation(
                                out=pexp, in_=st, func=AF.Exp, scale=0.125
                            )
                            for j in range(2):
                                nc.tensor.matmul(
                                    ctx[:, bass.ts(j, 512)],
                                    Vt[:, sk, h, :],
                                    pexp[:, bass.ts(j, 512)],
                                    start=(sk == 0),
                                    stop=(sk == NST - 1),
                                )
                        # copy PSUM out fast to release the ctx bank for the
                        # next head; normalize from SBUF off the critical path.
                        # The sums row is DMA-reshaped across 128 partitions so
                        # the (8 cyc/elem) reciprocal runs 128-wide.
                        ctxc = small.tile([65, 1024], F32, tag="ctxc")
                        nc.vector.tensor_copy(out=ctxc, in_=ctx[:, :])
                        rr = small.tile([128, 8], F32, tag="rr")
                        nc.sync.dma_start(out=rr, in_=ctxc[64:65, :])
                        rrv = small.tile([128, 8], F32, tag="rrv")
                        nc.vector.reciprocal(out=rrv, in_=rr)
                        rinvrow = small.tile([1, 1024], F32, tag="rinvrow")
                        nc.sync.dma_start(out=rinvrow, in_=rrv)
                        rbc = small.tile([64, 1024], F32, tag="rbc")
                        nc.gpsimd.partition_broadcast(rbc, rinvrow)
                        nc.vector.tensor_mul(
                            out=ctxn[po : po + 64, ht, q0 : q0 + 1024],
                            in0=ctxc[0:64, :],
                            in1=rbc,
                        )
                    # output projection for this q-pair
                    for dot in range(8):
                        for j in range(2):
                            pso = ps_o.tile([128, 512], F32, tag="po")
                            for kt in range(NHT):
                                nc.tensor.matmul(
                                    pso[:, :],
                                    wo_sb[:, kt, bass.ts(dot, 128)],
                                    ctxn[:, kt, q0 + j * 512 : q0 + (j + 1) * 512],
                                    start=(kt == 0),
                                    stop=(kt == NHT - 1),
                                )
                            osb = outp.tile([128, 512], F32, tag="osb")
                            nc.vector.tensor_copy(out=osb, in_=pso)
                            nc.sync.dma_start(
                                out=ot_v[dot, :, q0 + j * 512 : q0 + (j + 1) * 512],
                                in_=osb,
                            )

    nc.compile()
    return nc


_NC_CACHE = None


def _get_nc():
    global _NC_CACHE
    if _NC_CACHE is None:
        _NC_CACHE = build_nc()
    return _NC_CACHE


def kernel(q, k, v, Wq, bq, Wk, bk, Wv, bv, Wo, bo):
    q = np.asarray(q, np.float32)
    k = np.asarray(k, np.float32)
    v = np.asarray(v, np.float32)
    Wq = np.asarray(Wq, np.float32)
    Wk = np.asarray(Wk, np.float32)
    Wv = np.asarray(Wv, np.float32)
    Wo = np.asarray(Wo, np.float32)
    bq = np.asarray(bq, np.float32)
    bk = np.asarray(bk, np.float32)
    bv = np.asarray(bv, np.float32)
    bo = np.asarray(bo, np.float32)

    nc = _get_nc()
    in_maps = []
    for core in range(N_CORES):
        b, hg = core // 2, core % 2
        csl = slice(hg * DHT, (hg + 1) * DHT)
        in_maps.append(
            {
                "xq_t": np.ascontiguousarray(q[b].T),
                "xk_t": np.ascontiguousarray(k[b].T),
                "xv_t": np.ascontiguousarray(v[b].T),
                "wq": np.ascontiguousarray(Wq[:, csl]),
                "wk": np.ascontiguousarray(Wk[:, csl]),
                "wv": np.ascontiguousarray(Wv[:, csl]),
                "wo": np.ascontiguousarray(Wo[csl, :]),
                "bq": np.ascontiguousarray(bq[csl]),
                "bk": np.ascontiguousarray(bk[csl]),
                "bv": np.ascontiguousarray(bv[csl]),
            }
        )
    res = run_bass_kernel_spmd(nc, in_maps, list(range(N_CORES)))
    out = np.empty((B, S, D), np.float32)
    for b in range(B):
        o_t = res.results[2 * b]["o_t"] + res.results[2 * b + 1]["o_t"]
        out[b] = o_t.T + bo
    return out


# revision 12
# speedup vs baseline: 1.8401x; 1.3650x over previous
"""Multi-head attention (B=4, S=2048, D=1024, H=16) on 8 trn2 NeuronCores.

Sharding: (batch, head-group) -> 8 shards of (1 batch x 8 heads). Zero
cross-core communication: each core computes Q/K/V projections for its 8
heads, full attention over S=2048, and a partial output projection
(row-split Wo); the host sums the two head-group partials per batch.

Layout strategy: the host feeds per-batch inputs pre-transposed ([D, S]) so
every matmul's contraction dim lands on SBUF partitions without any
on-device transposes. The kernel keeps everything in "feature-major" form:
  Q^T, K^T: [dh, s]  -> scores^T[sk, q]  (softmax along partitions is
  avoided via an appended ones-column on V, which makes the PV matmul emit
  the softmax denominator as an extra output row)
  V: natural [s, dh] (+ ones col) -> ctx^T[dh(+1), q]
  out^T[do, q] = Wo_slice^T-contract(ctx^T / rowsum)

Matmul operands are bf16 (fp32 PSUM accumulation). bf16 halves DMA volume
and, unlike fp32/f32r, supports fast-weight-load + background-buffer
overlap, so the per-matmul LDWEIGHTS cost hides under the previous matmul
(f32r must self-load weights serially, measured ~2x slower end-to-end).
"""

import numpy as np

import concourse.bass as bass
import concourse.tile as tile
from concourse import bacc, mybir
from concourse.bass_utils import run_bass_kernel_spmd

F32 = mybir.dt.float32
BF16 = mybir.dt.bfloat16
AF = mybir.ActivationFunctionType

B, S, D = 4, 2048, 1024
HPC = 8          # heads per core
DHT = 512        # head dims per core (8 * 64)
NDT = D // 128   # 8 d-tiles (contraction tiles for projections)
NHT = DHT // 128  # 4 dh-tiles
NST = S // 128   # 16 s-tiles
NSB = S // 512   # 4 s-blocks
N_CORES = 8


def build_nc():
    nc = bacc.Bacc(None, target_bir_lowering=False)

    xq = nc.declare_dram_parameter("xq_t", [D, S], BF16, isOutput=False)
    xk = nc.declare_dram_parameter("xk_t", [D, S], BF16, isOutput=False)
    xv = nc.declare_dram_parameter("xv_t", [D, S], BF16, isOutput=False)
    wq = nc.declare_dram_parameter("wq", [D, DHT], BF16, isOutput=False)
    wk = nc.declare_dram_parameter("wk", [D, DHT], BF16, isOutput=False)
    wv = nc.declare_dram_parameter("wv", [D, DHT], BF16, isOutput=False)
    wo = nc.declare_dram_parameter("wo", [DHT, D], BF16, isOutput=False)
    bq = nc.declare_dram_parameter("bq", [DHT], F32, isOutput=False)
    bk = nc.declare_dram_parameter("bk", [DHT], F32, isOutput=False)
    bv = nc.declare_dram_parameter("bv", [DHT], F32, isOutput=False)
    ot = nc.declare_dram_parameter("o_t", [D, S], F32, isOutput=True)

    # DRAM views tiled to 128 partitions
    xq_v = xq.rearrange("(t p) s -> p t s", p=128)
    xk_v = xk.rearrange("(t p) s -> p t s", p=128)
    xv_v = xv.rearrange("(t p) s -> p t s", p=128)
    wq_v = wq.rearrange("(t p) n -> p t n", p=128)
    wk_v = wk.rearrange("(t p) n -> p t n", p=128)
    wv_v = wv.rearrange("(t p) n -> p t n", p=128)
    wo_v = wo.rearrange("(t p) n -> p t n", p=128)
    ot_v = ot.rearrange("(t p) s -> t p s", p=128)

    with tile.TileContext(nc) as tc:
        with (
            tc.tile_pool(name="persist", bufs=1) as persist,
            tc.tile_pool(name="outp", bufs=4) as outp,
            tc.tile_pool(name="ps_big", bufs=2, space="PSUM") as ps_big,
            tc.tile_pool(name="ps_ctx", bufs=1, space="PSUM") as ps_ctx,
            tc.tile_pool(name="ps_o", bufs=2, space="PSUM") as ps_o,
        ):
            KT = persist.tile([128, NHT, S], BF16)        # K^T  [dh, s]
            QT = persist.tile([128, NHT, S], BF16)        # Q^T  [dh, s]
            Vt = persist.tile([128, NST, HPC, 65], BF16)  # V natural + ones col
            bq_sb = persist.tile([128, NHT], F32)
            bk_sb = persist.tile([128, NHT], F32)
            bv_bc = persist.tile([128, HPC, 64], F32)

            nc.sync.dma_start(out=bq_sb, in_=bq.rearrange("(t p) -> p t", p=128))
            nc.sync.dma_start(out=bk_sb, in_=bk.rearrange("(t p) -> p t", p=128))
            nc.sync.dma_start(
                out=bv_bc,
                in_=bv.rearrange("(h d) -> h d", d=64).partition_broadcast(128),
            )
            # ones column for the PV matmul's softmax-denominator row
            nc.vector.memset(Vt[:, :, :, 64:65], 1.0)

            # ---------------- projections ----------------
            with (
                tc.tile_pool(name="w3", bufs=1) as w3,
                tc.tile_pool(name="xs", bufs=3) as xs,
            ):
                wq_sb = w3.tile([128, NDT, DHT], BF16)
                wk_sb = w3.tile([128, NDT, DHT], BF16)
                wv_sb = w3.tile([128, NDT, DHT], BF16)
                for dt in range(NDT):  # split DMAs across queues
                    nc.sync.dma_start(out=wq_sb[:, dt, :], in_=wq_v[:, dt, :])
                    nc.sync.dma_start(out=wk_sb[:, dt, :], in_=wk_v[:, dt, :])
                    nc.sync.dma_start(out=wv_sb[:, dt, :], in_=wv_v[:, dt, :])

                for xv_ap, w_sb, dst, b_sb in (
                    (xk_v, wk_sb, KT, bk_sb),
                    (xv_v, wv_sb, None, None),   # V handled below (natural layout)
                    (xq_v, wq_sb, QT, bq_sb),
                ):
                    for sb in range(NSB):
                        ssl = slice(sb * 512, (sb + 1) * 512)
                        xst = xs.tile([128, NDT, 512], BF16, tag="xs")
                        for dt in range(NDT):
                            nc.sync.dma_start(
                                out=xst[:, dt, :], in_=xv_ap[:, dt, ssl]
                            )
                        if dst is None:
                            # V projection, natural layout: the X^T tile is
                            # stationary so out[s-tile, dh] has s on partitions
                            for su in range(4):
                                pso = ps_o.tile([128, 512], F32, tag="po")
                                for dt in range(NDT):
                                    nc.tensor.matmul(
                                        pso[:, :],
                                        xst[:, dt, bass.ts(su, 128)],
                                        wv_sb[:, dt, :],
                                        start=(dt == 0),
                                        stop=(dt == NDT - 1),
                                    )
                                stile = sb * 4 + su
                                nc.vector.tensor_add(
                                    out=Vt[:, stile, :, 0:64],
                                    in0=pso.rearrange("p (h d) -> p h d", d=64),
                                    in1=bv_bc,
                                )
                        else:
                            # K^T / Q^T: out[dh-tile, s-blk] = W^T-contract X^T
                            for ht in range(NHT):
                                ps = ps_o.tile([128, 512], F32, tag="po")
                                for dt in range(NDT):
                                    nc.tensor.matmul(
                                        ps[:, :],
                                        w_sb[:, dt, bass.ts(ht, 128)],
                                        xst[:, dt, :],
                                        start=(dt == 0),
                                        stop=(dt == NDT - 1),
                                    )
                                # DVE (not ACT) so the scalar engine stays
                                # free for the softmax exps
                                nc.vector.tensor_scalar_add(
                                    out=dst[:, ht, ssl],
                                    in0=ps[:, :],
                                    scalar1=b_sb[:, ht : ht + 1],
                                )

            # ---------------- attention + output projection ----------------
            with (
                tc.tile_pool(name="wo_p", bufs=1) as wo_p,
                tc.tile_pool(name="ctxn_p", bufs=1) as ctxn_p,
                tc.tile_pool(name="pexp_p", bufs=3) as pexp_p,
                tc.tile_pool(name="small", bufs=2) as small,
            ):
                wo_sb = wo_p.tile([128, NHT, D], BF16)
                for kt in range(NHT):
                    nc.sync.dma_start(out=wo_sb[:, kt, :], in_=wo_v[:, kt, :])
                ctxn = ctxn_p.tile([128, NHT, S], BF16)

                for qp in range(2):  # q column pairs of 1024
                    q0 = qp * 1024
                    for h in range(HPC):
                        po = 64 * (h % 2)
                        ht = h // 2
                        ctx = ps_ctx.tile([65, 1024], F32, tag="ctx")
                        for sk in range(NST):
                            st = ps_big.tile([128, 1024], F32, tag="big")
                            for j in range(2):
                                nc.tensor.matmul(
                                    st[:, bass.ts(j, 512)],
                                    KT[po : po + 64, ht, bass.ts(sk, 128)],
                                    QT[po : po + 64, ht, q0 + j * 512 : q0 + (j + 1) * 512],
                                    start=True,
                                    stop=True,
                                )
                            pexp = pexp_p.tile([128, 1024], BF16, tag="pexp")
                            nc.scalar.activation(
                                out=pexp, in_=st, func=AF.Exp, scale=0.125
                            )
                            for j in range(2):
                                nc.tensor.matmul(
                                    ctx[:, bass.ts(j, 512)],
                                    Vt[:, sk, h, :],
                                    pexp[:, bass.ts(j, 512)],
                                    start=(sk == 0),
                                    stop=(sk == NST - 1),
                                )
                        # copy PSUM out fast to release the ctx bank for the
                        # next head; normalize from SBUF off the critical path.
                        # The sums row is DMA-reshaped across 128 partitions so
                        # the (8 cyc/elem) reciprocal runs 128-wide.
                        ctxc = small.tile([65, 1024], F32, tag="ctxc")
                        nc.vector.tensor_copy(out=ctxc, in_=ctx[:, :])
                        rr = small.tile([128, 8], F32, tag="rr")
                        nc.sync.dma_start(out=rr, in_=ctxc[64:65, :])
                        rrv = small.tile([128, 8], F32, tag="rrv")
                        nc.vector.reciprocal(out=rrv, in_=rr)
                        rinvrow = small.tile([1, 1024], F32, tag="rinvrow")
                        nc.sync.dma_start(out=rinvrow, in_=rrv)
                        rbc = small.tile([64, 1024], F32, tag="rbc")
                        nc.gpsimd.partition_broadcast(rbc, rinvrow)
                        nc.vector.tensor_mul(
                            out=ctxn[po : po + 64, ht, q0 : q0 + 1024],
                            in0=ctxc[0:64, :],
                            in1=rbc,
                        )
                    # output projection for this q-pair
                    for dot in range(8):
                        for j in range(2):
                            pso = ps_o.tile([128, 512], F32, tag="po")
                            for kt in range(NHT):
                                nc.tensor.matmul(
                                    pso[:, :],
                                    wo_sb[:, kt, bass.ts(dot, 128)],
                                    ctxn[:, kt, q0 + j * 512 : q0 + (j + 1) * 512],
                                    start=(kt == 0),
                                    stop=(kt == NHT - 1),
                                )
                            osb = outp.tile([128, 512], F32, tag="osb")
                            nc.vector.tensor_copy(out=osb, in_=pso)
                            nc.sync.dma_start(
                                out=ot_v[dot, :, q0 + j * 512 : q0 + (j + 1) * 512],
                                in_=osb,
                            )

    nc.compile()
    return nc


_NC_CACHE = None


def _get_nc():
    global _NC_CACHE
    if _NC_CACHE is None:
        _NC_CACHE = build_nc()
    return _NC_CACHE


def make_in_maps(q, k, v, Wq, bq, Wk, bk, Wv, bv, Wo):
    import ml_dtypes

    bf = ml_dtypes.bfloat16
    in_maps = []
    for core in range(N_CORES):
        b, hg = core // 2, core % 2
        csl = slice(hg * DHT, (hg + 1) * DHT)
        in_maps.append(
            {
                "xq_t": np.ascontiguousarray(q[b].T).astype(bf),
                "xk_t": np.ascontiguousarray(k[b].T).astype(bf),
                "xv_t": np.ascontiguousarray(v[b].T).astype(bf),
                "wq": np.ascontiguousarray(Wq[:, csl]).astype(bf),
                "wk": np.ascontiguousarray(Wk[:, csl]).astype(bf),
                "wv": np.ascontiguousarray(Wv[:, csl]).astype(bf),
                "wo": np.ascontiguousarray(Wo[csl, :]).astype(bf),
                "bq": np.ascontiguousarray(bq[csl]).astype(np.float32),
                "bk": np.ascontiguousarray(bk[csl]).astype(np.float32),
                "bv": np.ascontiguousarray(bv[csl]).astype(np.float32),
            }
        )
    return in_maps


def kernel(q, k, v, Wq, bq, Wk, bk, Wv, bv, Wo, bo):
    q = np.asarray(q, np.float32)
    k = np.asarray(k, np.float32)
    v = np.asarray(v, np.float32)
    Wq = np.asarray(Wq, np.float32)
    Wk = np.asarray(Wk, np.float32)
    Wv = np.asarray(Wv, np.float32)
    Wo = np.asarray(Wo, np.float32)
    bq = np.asarray(bq, np.float32)
    bk = np.asarray(bk, np.float32)
    bv = np.asarray(bv, np.float32)
    bo = np.asarray(bo, np.float32)

    nc = _get_nc()
    in_maps = make_in_maps(q, k, v, Wq, bq, Wk, bk, Wv, bv, Wo)
    res = run_bass_kernel_spmd(nc, in_maps, list(range(N_CORES)))
    out = np.empty((B, S, D), np.float32)
    for b in range(B):
        o_t = res.results[2 * b]["o_t"] + res.results[2 * b + 1]["o_t"]
        out[b] = o_t.T + bo
    return out


# revision 13
# speedup vs baseline: 1.8593x; 1.0105x over previous
"""Multi-head attention (B=4, S=2048, D=1024, H=16) on 8 trn2 NeuronCores.

Sharding: (batch, head-group) -> 8 shards of (1 batch x 8 heads). Zero
cross-core communication: each core computes Q/K/V projections for its 8
heads, full attention over S=2048, and a partial output projection
(row-split Wo); the host sums the two head-group partials per batch.

Layout strategy: the host feeds per-batch inputs pre-transposed ([D, S]) so
every matmul's contraction dim lands on SBUF partitions without any
on-device transposes. The kernel keeps everything in "feature-major" form:
  Q^T, K^T: [dh, s]  -> scores^T[sk, q]  (softmax along partitions is
  avoided via an appended ones-column on V, which makes the PV matmul emit
  the softmax denominator as an extra output row)
  V: natural [s, dh] (+ ones col) -> ctx^T[dh(+1), q]
  out^T[do, q] = Wo_slice^T-contract(ctx^T / rowsum)

Matmul operands are fp16 (fp32 PSUM accumulation; the PE multiplies
bf16/fp16 at FP22 internally, so fp16's 10 mantissa bits survive). fp16
halves DMA volume and, unlike fp32/f32r, supports fast-weight-load +
background-buffer overlap, so the per-matmul LDWEIGHTS cost hides under
the previous matmul (f32r must self-load weights serially, measured ~2x
slower end-to-end). Simulated pipeline accuracy: ~7e-4 rel.
"""

import numpy as np

import concourse.bass as bass
import concourse.tile as tile
from concourse import bacc, mybir
from concourse.bass_utils import run_bass_kernel_spmd

F32 = mybir.dt.float32
F16 = mybir.dt.float16
AF = mybir.ActivationFunctionType

B, S, D = 4, 2048, 1024
HPC = 8          # heads per core
DHT = 512        # head dims per core (8 * 64)
NDT = D // 128   # 8 d-tiles (contraction tiles for projections)
NHT = DHT // 128  # 4 dh-tiles
NST = S // 128   # 16 s-tiles
NSB = S // 512   # 4 s-blocks
N_CORES = 8


def build_nc():
    nc = bacc.Bacc(None, target_bir_lowering=False)

    xq = nc.declare_dram_parameter("xq_t", [D, S], F16, isOutput=False)
    xk = nc.declare_dram_parameter("xk_t", [D, S], F16, isOutput=False)
    xv = nc.declare_dram_parameter("xv_t", [D, S], F16, isOutput=False)
    wq = nc.declare_dram_parameter("wq", [D, DHT], F16, isOutput=False)
    wk = nc.declare_dram_parameter("wk", [D, DHT], F16, isOutput=False)
    wv = nc.declare_dram_parameter("wv", [D, DHT], F16, isOutput=False)
    wo = nc.declare_dram_parameter("wo", [DHT, D], F16, isOutput=False)
    bq = nc.declare_dram_parameter("bq", [DHT], F32, isOutput=False)
    bk = nc.declare_dram_parameter("bk", [DHT], F32, isOutput=False)
    bv = nc.declare_dram_parameter("bv", [DHT], F32, isOutput=False)
    ot = nc.declare_dram_parameter("o_t", [D, S], F32, isOutput=True)

    # DRAM views tiled to 128 partitions
    xq_v = xq.rearrange("(t p) s -> p t s", p=128)
    xk_v = xk.rearrange("(t p) s -> p t s", p=128)
    xv_v = xv.rearrange("(t p) s -> p t s", p=128)
    wq_v = wq.rearrange("(t p) n -> p t n", p=128)
    wk_v = wk.rearrange("(t p) n -> p t n", p=128)
    wv_v = wv.rearrange("(t p) n -> p t n", p=128)
    wo_v = wo.rearrange("(t p) n -> p t n", p=128)
    ot_v = ot.rearrange("(t p) s -> t p s", p=128)

    with tile.TileContext(nc) as tc:
        with (
            tc.tile_pool(name="persist", bufs=1) as persist,
            tc.tile_pool(name="outp", bufs=4) as outp,
            tc.tile_pool(name="ps_big", bufs=2, space="PSUM") as ps_big,
            tc.tile_pool(name="ps_ctx", bufs=1, space="PSUM") as ps_ctx,
            tc.tile_pool(name="ps_o", bufs=2, space="PSUM") as ps_o,
        ):
            KT = persist.tile([128, NHT, S], F16)        # K^T  [dh, s]
            QT = persist.tile([128, NHT, S], F16)        # Q^T  [dh, s]
            Vt = persist.tile([128, NST, HPC, 65], F16)  # V natural + ones col
            bq_sb = persist.tile([128, NHT], F32)
            bk_sb = persist.tile([128, NHT], F32)
            bv_bc = persist.tile([128, HPC, 64], F32)

            nc.sync.dma_start(out=bq_sb, in_=bq.rearrange("(t p) -> p t", p=128))
            nc.sync.dma_start(out=bk_sb, in_=bk.rearrange("(t p) -> p t", p=128))
            nc.sync.dma_start(
                out=bv_bc,
                in_=bv.rearrange("(h d) -> h d", d=64).partition_broadcast(128),
            )
            # ones column for the PV matmul's softmax-denominator row
            nc.vector.memset(Vt[:, :, :, 64:65], 1.0)

            # ---------------- projections ----------------
            with (
                tc.tile_pool(name="w3", bufs=1) as w3,
                tc.tile_pool(name="xs", bufs=3) as xs,
            ):
                wq_sb = w3.tile([128, NDT, DHT], F16)
                wk_sb = w3.tile([128, NDT, DHT], F16)
                wv_sb = w3.tile([128, NDT, DHT], F16)
                for dt in range(NDT):  # split DMAs across queues
                    nc.sync.dma_start(out=wq_sb[:, dt, :], in_=wq_v[:, dt, :])
                    nc.sync.dma_start(out=wk_sb[:, dt, :], in_=wk_v[:, dt, :])
                    nc.sync.dma_start(out=wv_sb[:, dt, :], in_=wv_v[:, dt, :])

                for xv_ap, w_sb, dst, b_sb in (
                    (xk_v, wk_sb, KT, bk_sb),
                    (xv_v, wv_sb, None, None),   # V handled below (natural layout)
                    (xq_v, wq_sb, QT, bq_sb),
                ):
                    for sb in range(NSB):
                        ssl = slice(sb * 512, (sb + 1) * 512)
                        xst = xs.tile([128, NDT, 512], F16, tag="xs")
                        for dt in range(NDT):
                            nc.sync.dma_start(
                                out=xst[:, dt, :], in_=xv_ap[:, dt, ssl]
                            )
                        if dst is None:
                            # V projection, natural layout: the X^T tile is
                            # stationary so out[s-tile, dh] has s on partitions
                            for su in range(4):
                                pso = ps_o.tile([128, 512], F32, tag="po")
                                for dt in range(NDT):
                                    nc.tensor.matmul(
                                        pso[:, :],
                                        xst[:, dt, bass.ts(su, 128)],
                                        wv_sb[:, dt, :],
                                        start=(dt == 0),
                                        stop=(dt == NDT - 1),
                                    )
                                stile = sb * 4 + su
                                nc.vector.tensor_add(
                                    out=Vt[:, stile, :, 0:64],
                                    in0=pso.rearrange("p (h d) -> p h d", d=64),
                                    in1=bv_bc,
                                )
                        else:
                            # K^T / Q^T: out[dh-tile, s-blk] = W^T-contract X^T
                            for ht in range(NHT):
                                ps = ps_o.tile([128, 512], F32, tag="po")
                                for dt in range(NDT):
                                    nc.tensor.matmul(
                                        ps[:, :],
                                        w_sb[:, dt, bass.ts(ht, 128)],
                                        xst[:, dt, :],
                                        start=(dt == 0),
                                        stop=(dt == NDT - 1),
                                    )
                                # DVE (not ACT) so the scalar engine stays
                                # free for the softmax exps
                                nc.vector.tensor_scalar_add(
                                    out=dst[:, ht, ssl],
                                    in0=ps[:, :],
                                    scalar1=b_sb[:, ht : ht + 1],
                                )

            # ---------------- attention + output projection ----------------
            with (
                tc.tile_pool(name="wo_p", bufs=1) as wo_p,
                tc.tile_pool(name="ctxn_p", bufs=1) as ctxn_p,
                tc.tile_pool(name="pexp_p", bufs=3) as pexp_p,
                tc.tile_pool(name="small", bufs=2) as small,
            ):
                wo_sb = wo_p.tile([128, NHT, D], F16)
                for kt in range(NHT):
                    nc.sync.dma_start(out=wo_sb[:, kt, :], in_=wo_v[:, kt, :])
                ctxn = ctxn_p.tile([128, NHT, S], F16)

                for qp in range(2):  # q column pairs of 1024
                    q0 = qp * 1024
                    for h in range(HPC):
                        po = 64 * (h % 2)
                        ht = h // 2
                        ctx = ps_ctx.tile([65, 1024], F32, tag="ctx")
                        for sk in range(NST):
                            st = ps_big.tile([128, 1024], F32, tag="big")
                            for j in range(2):
                                nc.tensor.matmul(
                                    st[:, bass.ts(j, 512)],
                                    KT[po : po + 64, ht, bass.ts(sk, 128)],
                                    QT[po : po + 64, ht, q0 + j * 512 : q0 + (j + 1) * 512],
                                    start=True,
                                    stop=True,
                                )
                            pexp = pexp_p.tile([128, 1024], F16, tag="pexp")
                            nc.scalar.activation(
                                out=pexp, in_=st, func=AF.Exp, scale=0.125
                            )
                            for j in range(2):
                                nc.tensor.matmul(
                                    ctx[:, bass.ts(j, 512)],
                                    Vt[:, sk, h, :],
                                    pexp[:, bass.ts(j, 512)],
                                    start=(sk == 0),
                                    stop=(sk == NST - 1),
                                )
                        # copy PSUM out fast to release the ctx bank for the
                        # next head; normalize from SBUF off the critical path.
                        # The sums row is DMA-reshaped across 128 partitions so
                        # the (8 cyc/elem) reciprocal runs 128-wide.
                        ctxc = small.tile([65, 1024], F32, tag="ctxc")
                        nc.vector.tensor_copy(out=ctxc, in_=ctx[:, :])
                        rr = small.tile([128, 8], F32, tag="rr")
                        nc.sync.dma_start(out=rr, in_=ctxc[64:65, :])
                        rrv = small.tile([128, 8], F32, tag="rrv")
                        nc.vector.reciprocal(out=rrv, in_=rr)
                        rinvrow = small.tile([1, 1024], F32, tag="rinvrow")
                        nc.sync.dma_start(out=rinvrow, in_=rrv)
                        rbc = small.tile([64, 1024], F32, tag="rbc")
                        nc.gpsimd.partition_broadcast(rbc, rinvrow)
                        nc.vector.tensor_mul(
                            out=ctxn[po : po + 64, ht, q0 : q0 + 1024],
                            in0=ctxc[0:64, :],
                            in1=rbc,
                        )
                    # output projection for this q-pair
                    for dot in range(8):
                        for j in range(2):
                            pso = ps_o.tile([128, 512], F32, tag="po")
                            for kt in range(NHT):
                                nc.tensor.matmul(
                                    pso[:, :],
                                    wo_sb[:, kt, bass.ts(dot, 128)],
                                    ctxn[:, kt, q0 + j * 512 : q0 + (j + 1) * 512],
                                    start=(kt == 0),
                                    stop=(kt == NHT - 1),
                                )
                            osb = outp.tile([128, 512], F32, tag="osb")
                            nc.vector.tensor_copy(out=osb, in_=pso)
                            nc.sync.dma_start(
                                out=ot_v[dot, :, q0 + j * 512 : q0 + (j + 1) * 512],
                                in_=osb,
                            )

    nc.compile()
    return nc


_NC_CACHE = None


def _get_nc():
    global _NC_CACHE
    if _NC_CACHE is None:
        _NC_CACHE = build_nc()
    return _NC_CACHE


def make_in_maps(q, k, v, Wq, bq, Wk, bk, Wv, bv, Wo):
    bf = np.float16
    in_maps = []
    for core in range(N_CORES):
        b, hg = core // 2, core % 2
        csl = slice(hg * DHT, (hg + 1) * DHT)
        in_maps.append(
            {
                "xq_t": np.ascontiguousarray(q[b].T).astype(bf),
                "xk_t": np.ascontiguousarray(k[b].T).astype(bf),
                "xv_t": np.ascontiguousarray(v[b].T).astype(bf),
                "wq": np.ascontiguousarray(Wq[:, csl]).astype(bf),
                "wk": np.ascontiguousarray(Wk[:, csl]).astype(bf),
                "wv": np.ascontiguousarray(Wv[:, csl]).astype(bf),
                "wo": np.ascontiguousarray(Wo[csl, :]).astype(bf),
                "bq": np.ascontiguousarray(bq[csl]).astype(np.float32),
                "bk": np.ascontiguousarray(bk[csl]).astype(np.float32),
                "bv": np.ascontiguousarray(bv[csl]).astype(np.float32),
            }
        )
    return in_maps


def kernel(q, k, v, Wq, bq, Wk, bk, Wv, bv, Wo, bo):
    q = np.asarray(q, np.float32)
    k = np.asarray(k, np.float32)
    v = np.asarray(v, np.float32)
    Wq = np.asarray(Wq, np.float32)
    Wk = np.asarray(Wk, np.float32)
    Wv = np.asarray(Wv, np.float32)
    Wo = np.asarray(Wo, np.float32)
    bq = np.asarray(bq, np.float32)
    bk = np.asarray(bk, np.float32)
    bv = np.asarray(bv, np.float32)
    bo = np.asarray(bo, np.float32)

    nc = _get_nc()
    in_maps = make_in_maps(q, k, v, Wq, bq, Wk, bk, Wv, bv, Wo)
    res = run_bass_kernel_spmd(nc, in_maps, list(range(N_CORES)))
    out = np.empty((B, S, D), np.float32)
    for b in range(B):
        o_t = res.results[2 * b]["o_t"] + res.results[2 * b + 1]["o_t"]
        out[b] = o_t.T + bo
    return out


# revision 14
# speedup vs baseline: 1.8779x; 1.0100x over previous
"""Multi-head attention (B=4, S=2048, D=1024, H=16) on 8 trn2 NeuronCores.

Sharding: (batch, head-group) -> 8 shards of (1 batch x 8 heads). Zero
cross-core communication: each core computes Q/K/V projections for its 8
heads, full attention over S=2048, and a partial output projection
(row-split Wo); the host sums the two head-group partials per batch.

Layout strategy: the host feeds per-batch inputs pre-transposed ([D, S]) so
every matmul's contraction dim lands on SBUF partitions without any
on-device transposes. The kernel keeps everything in "feature-major" form:
  Q^T, K^T: [dh, s]  -> scores^T[sk, q]  (softmax along partitions is
  avoided via an appended ones-column on V, which makes the PV matmul emit
  the softmax denominator as an extra output row)
  V: natural [s, dh] (+ ones col) -> ctx^T[dh(+1), q]
  out^T[do, q] = Wo_slice^T-contract(ctx^T / rowsum)

Matmul operands are fp16 (fp32 PSUM accumulation; the PE multiplies
bf16/fp16 at FP22 internally, so fp16's 10 mantissa bits survive). fp16
halves DMA volume and, unlike fp32/f32r, supports fast-weight-load +
background-buffer overlap, so the per-matmul LDWEIGHTS cost hides under
the previous matmul (f32r must self-load weights serially, measured ~2x
slower end-to-end). Simulated pipeline accuracy: ~7e-4 rel.
"""

import numpy as np

import concourse.bass as bass
import concourse.tile as tile
from concourse import bacc, mybir
from concourse.bass_utils import run_bass_kernel_spmd

F32 = mybir.dt.float32
F16 = mybir.dt.float16
AF = mybir.ActivationFunctionType

B, S, D = 4, 2048, 1024
HPC = 8          # heads per core
DHT = 512        # head dims per core (8 * 64)
NDT = D // 128   # 8 d-tiles (contraction tiles for projections)
NHT = DHT // 128  # 4 dh-tiles
NST = S // 128   # 16 s-tiles
NSB = S // 512   # 4 s-blocks
N_CORES = 8


def build_nc():
    nc = bacc.Bacc(None, target_bir_lowering=False)

    xq = nc.declare_dram_parameter("xq_t", [D, S], F16, isOutput=False)
    xk = nc.declare_dram_parameter("xk_t", [D, S], F16, isOutput=False)
    xv = nc.declare_dram_parameter("xv_t", [D, S], F16, isOutput=False)
    wq = nc.declare_dram_parameter("wq", [D, DHT], F16, isOutput=False)
    wk = nc.declare_dram_parameter("wk", [D, DHT], F16, isOutput=False)
    wv = nc.declare_dram_parameter("wv", [D, DHT], F16, isOutput=False)
    wo = nc.declare_dram_parameter("wo", [DHT, D], F16, isOutput=False)
    bq = nc.declare_dram_parameter("bq", [DHT], F32, isOutput=False)
    bk = nc.declare_dram_parameter("bk", [DHT], F32, isOutput=False)
    bv = nc.declare_dram_parameter("bv", [DHT], F32, isOutput=False)
    ot = nc.declare_dram_parameter("o_t", [D, S], F32, isOutput=True)

    # DRAM views tiled to 128 partitions
    xq_v = xq.rearrange("(t p) s -> p t s", p=128)
    xk_v = xk.rearrange("(t p) s -> p t s", p=128)
    xv_v = xv.rearrange("(t p) s -> p t s", p=128)
    wq_v = wq.rearrange("(t p) n -> p t n", p=128)
    wk_v = wk.rearrange("(t p) n -> p t n", p=128)
    wv_v = wv.rearrange("(t p) n -> p t n", p=128)
    wo_v = wo.rearrange("(t p) n -> p t n", p=128)
    ot_v = ot.rearrange("(t p) s -> t p s", p=128)

    with tile.TileContext(nc) as tc:
        with (
            tc.tile_pool(name="persist", bufs=1) as persist,
            tc.tile_pool(name="outp", bufs=4) as outp,
            tc.tile_pool(name="ps_big", bufs=2, space="PSUM") as ps_big,
            tc.tile_pool(name="ps_ctx", bufs=1, space="PSUM") as ps_ctx,
            tc.tile_pool(name="ps_o", bufs=2, space="PSUM") as ps_o,
        ):
            KT = persist.tile([128, NHT, S], F16)        # K^T  [dh, s]
            QT = persist.tile([128, NHT, S], F16)        # Q^T  [dh, s]
            Vt = persist.tile([128, NST, HPC, 65], F16)  # V natural + ones col
            bq_sb = persist.tile([128, NHT], F32)
            bk_sb = persist.tile([128, NHT], F32)
            bv_bc = persist.tile([128, HPC, 64], F32)

            nc.sync.dma_start(out=bq_sb, in_=bq.rearrange("(t p) -> p t", p=128))
            nc.sync.dma_start(out=bk_sb, in_=bk.rearrange("(t p) -> p t", p=128))
            nc.sync.dma_start(
                out=bv_bc,
                in_=bv.rearrange("(h d) -> h d", d=64).partition_broadcast(128),
            )
            # ones column for the PV matmul's softmax-denominator row
            nc.vector.memset(Vt[:, :, :, 64:65], 1.0)

            # ---------------- projections ----------------
            with (
                tc.tile_pool(name="w3", bufs=1) as w3,
                tc.tile_pool(name="xs", bufs=3) as xs,
            ):
                wq_sb = w3.tile([128, NDT, DHT], F16)
                wk_sb = w3.tile([128, NDT, DHT], F16)
                wv_sb = w3.tile([128, NDT, DHT], F16)
                for dt in range(NDT):  # split DMAs across queues; K first
                    nc.sync.dma_start(out=wk_sb[:, dt, :], in_=wk_v[:, dt, :])
                for dt in range(NDT):
                    nc.sync.dma_start(out=wv_sb[:, dt, :], in_=wv_v[:, dt, :])
                for dt in range(NDT):
                    nc.sync.dma_start(out=wq_sb[:, dt, :], in_=wq_v[:, dt, :])

                # s-block-major emission so attention (emitted after, lower
                # priority) can start once the first s-block of K/V/Q exists;
                # remaining projection matmuls fill PE slack during the
                # ACT-paced attention phase.
                for sb in range(NSB):
                    ssl = slice(sb * 512, (sb + 1) * 512)
                    for xv_ap, w_sb, dst, b_sb in (
                        (xk_v, wk_sb, KT, bk_sb),
                        (xv_v, wv_sb, None, None),  # V below (natural layout)
                        (xq_v, wq_sb, QT, bq_sb),
                    ):
                        xst = xs.tile([128, NDT, 512], F16, tag="xs")
                        for dt in range(NDT):
                            nc.sync.dma_start(
                                out=xst[:, dt, :], in_=xv_ap[:, dt, ssl]
                            )
                        if dst is None:
                            # V projection, natural layout: the X^T tile is
                            # stationary so out[s-tile, dh] has s on partitions
                            for su in range(4):
                                pso = ps_o.tile([128, 512], F32, tag="po")
                                for dt in range(NDT):
                                    nc.tensor.matmul(
                                        pso[:, :],
                                        xst[:, dt, bass.ts(su, 128)],
                                        wv_sb[:, dt, :],
                                        start=(dt == 0),
                                        stop=(dt == NDT - 1),
                                    )
                                stile = sb * 4 + su
                                nc.vector.tensor_add(
                                    out=Vt[:, stile, :, 0:64],
                                    in0=pso.rearrange("p (h d) -> p h d", d=64),
                                    in1=bv_bc,
                                )
                        else:
                            # K^T / Q^T: out[dh-tile, s-blk] = W^T-contract X^T
                            for ht in range(NHT):
                                ps = ps_o.tile([128, 512], F32, tag="po")
                                for dt in range(NDT):
                                    nc.tensor.matmul(
                                        ps[:, :],
                                        w_sb[:, dt, bass.ts(ht, 128)],
                                        xst[:, dt, :],
                                        start=(dt == 0),
                                        stop=(dt == NDT - 1),
                                    )
                                # DVE (not ACT) so the scalar engine stays
                                # free for the softmax exps
                                nc.vector.tensor_scalar_add(
                                    out=dst[:, ht, ssl],
                                    in0=ps[:, :],
                                    scalar1=b_sb[:, ht : ht + 1],
                                )

            # ---------------- attention + output projection ----------------
            with (
                tc.tile_pool(name="wo_p", bufs=1) as wo_p,
                tc.tile_pool(name="ctxn_p", bufs=1) as ctxn_p,
                tc.tile_pool(name="pexp_p", bufs=3) as pexp_p,
                tc.tile_pool(name="small", bufs=2) as small,
            ):
                wo_sb = wo_p.tile([128, NHT, D], F16)
                for kt in range(NHT):
                    nc.sync.dma_start(out=wo_sb[:, kt, :], in_=wo_v[:, kt, :])
                ctxn = ctxn_p.tile([128, NHT, S], F16)

                for qp in range(2):  # q column pairs of 1024
                    q0 = qp * 1024
                    for h in range(HPC):
                        po = 64 * (h % 2)
                        ht = h // 2
                        ctx = ps_ctx.tile([65, 1024], F32, tag="ctx")
                        for sk in range(NST):
                            st = ps_big.tile([128, 1024], F32, tag="big")
                            for j in range(2):
                                nc.tensor.matmul(
                                    st[:, bass.ts(j, 512)],
                                    KT[po : po + 64, ht, bass.ts(sk, 128)],
                                    QT[po : po + 64, ht, q0 + j * 512 : q0 + (j + 1) * 512],
                                    start=True,
                                    stop=True,
                                )
                            pexp = pexp_p.tile([128, 1024], F16, tag="pexp")
                            nc.scalar.activation(
                                out=pexp, in_=st, func=AF.Exp, scale=0.125
                            )
                            for j in range(2):
                                nc.tensor.matmul(
                                    ctx[:, bass.ts(j, 512)],
                                    Vt[:, sk, h, :],
                                    pexp[:, bass.ts(j, 512)],
                                    start=(sk == 0),
                                    stop=(sk == NST - 1),
                                )
                        # copy PSUM out fast to release the ctx bank for the
                        # next head; normalize from SBUF off the critical path.
                        # The sums row is DMA-reshaped across 128 partitions so
                        # the (8 cyc/elem) reciprocal runs 128-wide.
                        ctxc = small.tile([65, 1024], F32, tag="ctxc")
                        nc.vector.tensor_copy(out=ctxc, in_=ctx[:, :])
                        rr = small.tile([128, 8], F32, tag="rr")
                        nc.sync.dma_start(out=rr, in_=ctxc[64:65, :])
                        rrv = small.tile([128, 8], F32, tag="rrv")
                        nc.vector.reciprocal(out=rrv, in_=rr)
                        rinvrow = small.tile([1, 1024], F32, tag="rinvrow")
                        nc.sync.dma_start(out=rinvrow, in_=rrv)
                        rbc = small.tile([64, 1024], F32, tag="rbc")
                        nc.gpsimd.partition_broadcast(rbc, rinvrow)
                        nc.vector.tensor_mul(
                            out=ctxn[po : po + 64, ht, q0 : q0 + 1024],
                            in0=ctxc[0:64, :],
                            in1=rbc,
                        )
                    # output projection for this q-pair
                    for dot in range(8):
                        for j in range(2):
                            pso = ps_o.tile([128, 512], F32, tag="po")
                            for kt in range(NHT):
                                nc.tensor.matmul(
                                    pso[:, :],
                                    wo_sb[:, kt, bass.ts(dot, 128)],
                                    ctxn[:, kt, q0 + j * 512 : q0 + (j + 1) * 512],
                                    start=(kt == 0),
                                    stop=(kt == NHT - 1),
                                )
                            osb = outp.tile([128, 512], F32, tag="osb")
                            nc.vector.tensor_copy(out=osb, in_=pso)
                            nc.sync.dma_start(
                                out=ot_v[dot, :, q0 + j * 512 : q0 + (j + 1) * 512],
                                in_=osb,
                            )

    nc.compile()
    return nc


_NC_CACHE = None


def _get_nc():
    global _NC_CACHE
    if _NC_CACHE is None:
        _NC_CACHE = build_nc()
    return _NC_CACHE


def make_in_maps(q, k, v, Wq, bq, Wk, bk, Wv, bv, Wo):
    bf = np.float16
    in_maps = []
    for core in range(N_CORES):
        b, hg = core // 2, core % 2
        csl = slice(hg * DHT, (hg + 1) * DHT)
        in_maps.append(
            {
                "xq_t": np.ascontiguousarray(q[b].T).astype(bf),
                "xk_t": np.ascontiguousarray(k[b].T).astype(bf),
                "xv_t": np.ascontiguousarray(v[b].T).astype(bf),
                "wq": np.ascontiguousarray(Wq[:, csl]).astype(bf),
                "wk": np.ascontiguousarray(Wk[:, csl]).astype(bf),
                "wv": np.ascontiguousarray(Wv[:, csl]).astype(bf),
                "wo": np.ascontiguousarray(Wo[csl, :]).astype(bf),
                "bq": np.ascontiguousarray(bq[csl]).astype(np.float32),
                "bk": np.ascontiguousarray(bk[csl]).astype(np.float32),
                "bv": np.ascontiguousarray(bv[csl]).astype(np.float32),
            }
        )
    return in_maps


def kernel(q, k, v, Wq, bq, Wk, bk, Wv, bv, Wo, bo):
    q = np.asarray(q, np.float32)
    k = np.asarray(k, np.float32)
    v = np.asarray(v, np.float32)
    Wq = np.asarray(Wq, np.float32)
    Wk = np.asarray(Wk, np.float32)
    Wv = np.asarray(Wv, np.float32)
    Wo = np.asarray(Wo, np.float32)
    bq = np.asarray(bq, np.float32)
    bk = np.asarray(bk, np.float32)
    bv = np.asarray(bv, np.float32)
    bo = np.asarray(bo, np.float32)

    nc = _get_nc()
    in_maps = make_in_maps(q, k, v, Wq, bq, Wk, bk, Wv, bv, Wo)
    res = run_bass_kernel_spmd(nc, in_maps, list(range(N_CORES)))
    out = np.empty((B, S, D), np.float32)
    for b in range(B):
        o_t = res.results[2 * b]["o_t"] + res.results[2 * b + 1]["o_t"]
        out[b] = o_t.T + bo
    return out


# revision 16
# speedup vs baseline: 1.9154x; 1.0200x over previous
"""Multi-head attention (B=4, S=2048, D=1024, H=16) on 8 trn2 NeuronCores.

Sharding: (batch, head-group) -> 8 shards of (1 batch x 8 heads). Zero
cross-core communication: each core computes Q/K/V projections for its 8
heads, full attention over S=2048, and a partial output projection
(row-split Wo); the host sums the two head-group partials per batch.

Layout strategy: the host feeds per-batch inputs pre-transposed ([D, S]) so
every matmul's contraction dim lands on SBUF partitions without any
on-device transposes. The kernel keeps everything in "feature-major" form:
  Q^T, K^T: [dh, s]  -> scores^T[sk, q]  (softmax along partitions is
  avoided via an appended ones-column on V, which makes the PV matmul emit
  the softmax denominator as an extra output row)
  V: natural [s, dh] (+ ones col) -> ctx^T[dh(+1), q]
  out^T[do, q] = Wo_slice^T-contract(ctx^T / rowsum)

Matmul operands are fp16 (fp32 PSUM accumulation; the PE multiplies
bf16/fp16 at FP22 internally, so fp16's 10 mantissa bits survive). fp16
halves DMA volume and, unlike fp32/f32r, supports fast-weight-load +
background-buffer overlap, so the per-matmul LDWEIGHTS cost hides under
the previous matmul (f32r must self-load weights serially, measured ~2x
slower end-to-end). Simulated pipeline accuracy: ~7e-4 rel.
"""

import numpy as np

import concourse.bass as bass
import concourse.tile as tile
from concourse import bacc, mybir
from concourse.bass_utils import run_bass_kernel_spmd

F32 = mybir.dt.float32
F16 = mybir.dt.float16
AF = mybir.ActivationFunctionType

B, S, D = 4, 2048, 1024
HPC = 8          # heads per core
DHT = 512        # head dims per core (8 * 64)
NDT = D // 128   # 8 d-tiles (contraction tiles for projections)
NHT = DHT // 128  # 4 dh-tiles
NST = S // 128   # 16 s-tiles
NSB = S // 512   # 4 s-blocks
N_CORES = 8


def build_nc():
    nc = bacc.Bacc(None, target_bir_lowering=False)

    xq = nc.declare_dram_parameter("xq_t", [D, S], F16, isOutput=False)
    xk = nc.declare_dram_parameter("xk_t", [D, S], F16, isOutput=False)
    xv = nc.declare_dram_parameter("xv_t", [D, S], F16, isOutput=False)
    wq = nc.declare_dram_parameter("wq", [D, DHT], F16, isOutput=False)
    wk = nc.declare_dram_parameter("wk", [D, DHT], F16, isOutput=False)
    wv = nc.declare_dram_parameter("wv", [D, DHT], F16, isOutput=False)
    wo = nc.declare_dram_parameter("wo", [DHT, D], F16, isOutput=False)
    bq = nc.declare_dram_parameter("bq", [DHT], F32, isOutput=False)
    bk = nc.declare_dram_parameter("bk", [DHT], F32, isOutput=False)
    bv = nc.declare_dram_parameter("bv", [DHT], F32, isOutput=False)
    ot = nc.declare_dram_parameter("o_t", [D, S], F32, isOutput=True)

    # DRAM views tiled to 128 partitions
    xq_v = xq.rearrange("(t p) s -> p t s", p=128)
    xk_v = xk.rearrange("(t p) s -> p t s", p=128)
    xv_v = xv.rearrange("(t p) s -> p t s", p=128)
    wq_v = wq.rearrange("(t p) n -> p t n", p=128)
    wk_v = wk.rearrange("(t p) n -> p t n", p=128)
    wv_v = wv.rearrange("(t p) n -> p t n", p=128)
    wo_v = wo.rearrange("(t p) n -> p t n", p=128)
    ot_v = ot.rearrange("(t p) s -> t p s", p=128)

    with tile.TileContext(nc) as tc:
        with (
            tc.tile_pool(name="persist", bufs=1) as persist,
            tc.tile_pool(name="outp", bufs=4) as outp,
            tc.tile_pool(name="w3", bufs=1) as w3,
            tc.tile_pool(name="xs", bufs=3) as xs,
            tc.tile_pool(name="pexp_p", bufs=3) as pexp_p,
            tc.tile_pool(name="small", bufs=2) as small,
            tc.tile_pool(name="ps_big", bufs=2, space="PSUM") as ps_big,
            tc.tile_pool(name="ps_ctx", bufs=1, space="PSUM") as ps_ctx,
            tc.tile_pool(name="ps_o", bufs=2, space="PSUM") as ps_o,
        ):
            KT = persist.tile([128, NHT, S], F16)        # K^T  [dh, s]
            QT = persist.tile([128, NHT, S], F16)        # Q^T  [dh, s]
            Vt = persist.tile([128, NST, HPC, 65], F16)  # V natural + ones col
            ctxn = persist.tile([128, NHT, S], F16)      # normalized ctx^T
            wo_sb = persist.tile([128, NHT, D], F16)
            bq_sb = persist.tile([128, NHT], F32)
            bk_sb = persist.tile([128, NHT], F32)
            bv_bc = persist.tile([128, HPC, 64], F32)
            wq_sb = w3.tile([128, NDT, DHT], F16)
            wk_sb = w3.tile([128, NDT, DHT], F16)
            wv_sb = w3.tile([128, NDT, DHT], F16)

            nc.sync.dma_start(out=bq_sb, in_=bq.rearrange("(t p) -> p t", p=128))
            nc.sync.dma_start(out=bk_sb, in_=bk.rearrange("(t p) -> p t", p=128))
            nc.sync.dma_start(
                out=bv_bc,
                in_=bv.rearrange("(h d) -> h d", d=64).partition_broadcast(128),
            )
            # ones column for the PV matmul's softmax-denominator row
            nc.vector.memset(Vt[:, :, :, 64:65], 1.0)
            for dt in range(NDT):  # split DMAs across queues; K first
                nc.sync.dma_start(out=wk_sb[:, dt, :], in_=wk_v[:, dt, :])
            for dt in range(NDT):
                nc.sync.dma_start(out=wv_sb[:, dt, :], in_=wv_v[:, dt, :])
            for dt in range(NDT):
                nc.sync.dma_start(out=wq_sb[:, dt, :], in_=wq_v[:, dt, :])
            for kt in range(NHT):
                nc.sync.dma_start(out=wo_sb[:, kt, :], in_=wo_v[:, kt, :])

            def emit_proj(kind, sb):
                ssl = slice(sb * 512, (sb + 1) * 512)
                xv_ap = {"k": xk_v, "v": xv_v, "q": xq_v}[kind]
                xst = xs.tile([128, NDT, 512], F16, tag="xs")
                for dt in range(NDT):
                    nc.sync.dma_start(out=xst[:, dt, :], in_=xv_ap[:, dt, ssl])
                if kind == "v":
                    # V projection, natural layout: the X^T tile is
                    # stationary so out[s-tile, dh] has s on partitions
                    for su in range(4):
                        pso = ps_o.tile([128, 512], F32, tag="po")
                        for dt in range(NDT):
                            nc.tensor.matmul(
                                pso[:, :],
                                xst[:, dt, bass.ts(su, 128)],
                                wv_sb[:, dt, :],
                                start=(dt == 0),
                                stop=(dt == NDT - 1),
                            )
                        nc.vector.tensor_add(
                            out=Vt[:, sb * 4 + su, :, 0:64],
                            in0=pso.rearrange("p (h d) -> p h d", d=64),
                            in1=bv_bc,
                        )
                else:
                    w_sb = wk_sb if kind == "k" else wq_sb
                    dst = KT if kind == "k" else QT
                    b_sb = bk_sb if kind == "k" else bq_sb
                    # K^T / Q^T: out[dh-tile, s-blk] = W^T-contract X^T
                    for ht in range(NHT):
                        ps = ps_o.tile([128, 512], F32, tag="po")
                        for dt in range(NDT):
                            nc.tensor.matmul(
                                ps[:, :],
                                w_sb[:, dt, bass.ts(ht, 128)],
                                xst[:, dt, :],
                                start=(dt == 0),
                                stop=(dt == NDT - 1),
                            )
                        # DVE (not ACT) so the scalar engine stays
                        # free for the softmax exps
                        nc.vector.tensor_scalar_add(
                            out=dst[:, ht, ssl],
                            in0=ps[:, :],
                            scalar1=b_sb[:, ht : ht + 1],
                        )

            def emit_attention(qp):
                q0 = qp * 1024
                for h in range(HPC):
                    po = 64 * (h % 2)
                    ht = h // 2
                    ctx = ps_ctx.tile([65, 1024], F32, tag="ctx")
                    for sk in range(NST):
                        st = ps_big.tile([128, 1024], F32, tag="big")
                        for j in range(2):
                            nc.tensor.matmul(
                                st[:, bass.ts(j, 512)],
                                KT[po : po + 64, ht, bass.ts(sk, 128)],
                                QT[po : po + 64, ht, q0 + j * 512 : q0 + (j + 1) * 512],
                                start=True,
                                stop=True,
                            )
                        pexp = pexp_p.tile([128, 1024], F16, tag="pexp")
                        nc.scalar.activation(
                            out=pexp, in_=st, func=AF.Exp, scale=0.125
                        )
                        for j in range(2):
                            nc.tensor.matmul(
                                ctx[:, bass.ts(j, 512)],
                                Vt[:, sk, h, :],
                                pexp[:, bass.ts(j, 512)],
                                start=(sk == 0),
                                stop=(sk == NST - 1),
                            )
                    # copy PSUM out fast to release the ctx bank for the
                    # next head; normalize from SBUF off the critical path.
                    # The sums row is DMA-reshaped across 128 partitions so
                    # the (8 cyc/elem) reciprocal runs 128-wide.
                    ctxc = small.tile([65, 1024], F32, tag="ctxc")
                    nc.vector.tensor_copy(out=ctxc, in_=ctx[:, :])
                    rr = small.tile([128, 8], F32, tag="rr")
                    nc.sync.dma_start(out=rr, in_=ctxc[64:65, :])
                    rrv = small.tile([128, 8], F32, tag="rrv")
                    nc.vector.reciprocal(out=rrv, in_=rr)
                    rinvrow = small.tile([1, 1024], F32, tag="rinvrow")
                    nc.sync.dma_start(out=rinvrow, in_=rrv)
                    rbc = small.tile([64, 1024], F32, tag="rbc")
                    nc.gpsimd.partition_broadcast(rbc, rinvrow)
                    nc.vector.tensor_mul(
                        out=ctxn[po : po + 64, ht, q0 : q0 + 1024],
                        in0=ctxc[0:64, :],
                        in1=rbc,
                    )
                # output projection for this q-pair
                for dot in range(8):
                    for j in range(2):
                        pso = ps_o.tile([128, 512], F32, tag="po")
                        for kt in range(NHT):
                            nc.tensor.matmul(
                                pso[:, :],
                                wo_sb[:, kt, bass.ts(dot, 128)],
                                ctxn[:, kt, q0 + j * 512 : q0 + (j + 1) * 512],
                                start=(kt == 0),
                                stop=(kt == NHT - 1),
                            )
                        osb = outp.tile([128, 512], F32, tag="osb")
                        nc.vector.tensor_copy(out=osb, in_=pso)
                        nc.sync.dma_start(
                            out=ot_v[dot, :, q0 + j * 512 : q0 + (j + 1) * 512],
                            in_=osb,
                        )

            # All projections are emitted first (program order = dependency
            # order), with the blocks attention(qp0) needs first. attention(
            # qp0) is then emitted at scheduler priority 0: it preempts the
            # deferred projections as soon as each of its inputs is placed,
            # and the remaining projection matmuls fill the PE slack of the
            # ACT(exp)-paced attention phase.
            emit_proj("k", 0)
            emit_proj("v", 0)
            emit_proj("q", 0)
            emit_proj("q", 1)
            for sb in (1, 2, 3):
                emit_proj("k", sb)
                emit_proj("v", sb)
            emit_proj("q", 2)
            emit_proj("q", 3)
            with tc.high_priority():
                emit_attention(0)
            emit_attention(1)

    nc.compile()
    return nc


_NC_CACHE = None


def _get_nc():
    global _NC_CACHE
    if _NC_CACHE is None:
        _NC_CACHE = build_nc()
    return _NC_CACHE


def make_in_maps(q, k, v, Wq, bq, Wk, bk, Wv, bv, Wo):
    bf = np.float16
    in_maps = []
    for core in range(N_CORES):
        b, hg = core // 2, core % 2
        csl = slice(hg * DHT, (hg + 1) * DHT)
        in_maps.append(
            {
                "xq_t": np.ascontiguousarray(q[b].T).astype(bf),
                "xk_t": np.ascontiguousarray(k[b].T).astype(bf),
                "xv_t": np.ascontiguousarray(v[b].T).astype(bf),
                "wq": np.ascontiguousarray(Wq[:, csl]).astype(bf),
                "wk": np.ascontiguousarray(Wk[:, csl]).astype(bf),
                "wv": np.ascontiguousarray(Wv[:, csl]).astype(bf),
                "wo": np.ascontiguousarray(Wo[csl, :]).astype(bf),
                "bq": np.ascontiguousarray(bq[csl]).astype(np.float32),
                "bk": np.ascontiguousarray(bk[csl]).astype(np.float32),
                "bv": np.ascontiguousarray(bv[csl]).astype(np.float32),
            }
        )
    return in_maps


def kernel(q, k, v, Wq, bq, Wk, bk, Wv, bv, Wo, bo):
    q = np.asarray(q, np.float32)
    k = np.asarray(k, np.float32)
    v = np.asarray(v, np.float32)
    Wq = np.asarray(Wq, np.float32)
    Wk = np.asarray(Wk, np.float32)
    Wv = np.asarray(Wv, np.float32)
    Wo = np.asarray(Wo, np.float32)
    bq = np.asarray(bq, np.float32)
    bk = np.asarray(bk, np.float32)
    bv = np.asarray(bv, np.float32)
    bo = np.asarray(bo, np.float32)

    nc = _get_nc()
    in_maps = make_in_maps(q, k, v, Wq, bq, Wk, bk, Wv, bv, Wo)
    res = run_bass_kernel_spmd(nc, in_maps, list(range(N_CORES)))
    out = np.empty((B, S, D), np.float32)
    for b in range(B):
        o_t = res.results[2 * b]["o_t"] + res.results[2 * b + 1]["o_t"]
        out[b] = o_t.T + bo
    return out


# revision 17
# speedup vs baseline: 1.9439x; 1.0149x over previous
"""Multi-head attention (B=4, S=2048, D=1024, H=16) on 8 trn2 NeuronCores.

Sharding: (batch, head-group) -> 8 shards of (1 batch x 8 heads). Zero
cross-core communication: each core computes Q/K/V projections for its 8
heads, full attention over S=2048, and a partial output projection
(row-split Wo); the host sums the two head-group partials per batch.

Layout strategy: the host feeds per-batch inputs pre-transposed ([D, S]) so
every matmul's contraction dim lands on SBUF partitions without any
on-device transposes. The kernel keeps everything in "feature-major" form:
  Q^T, K^T: [dh, s]  -> scores^T[sk, q]  (softmax along partitions is
  avoided via an appended ones-column on V, which makes the PV matmul emit
  the softmax denominator as an extra output row)
  V: natural [s, dh] (+ ones col) -> ctx^T[dh(+1), q]
  out^T[do, q] = Wo_slice^T-contract(ctx^T / rowsum)

Matmul operands are fp16 (fp32 PSUM accumulation; the PE multiplies
bf16/fp16 at FP22 internally, so fp16's 10 mantissa bits survive). fp16
halves DMA volume and, unlike fp32/f32r, supports fast-weight-load +
background-buffer overlap, so the per-matmul LDWEIGHTS cost hides under
the previous matmul (f32r must self-load weights serially, measured ~2x
slower end-to-end). Simulated pipeline accuracy: ~7e-4 rel.
"""

import numpy as np

import concourse.bass as bass
import concourse.tile as tile
from concourse import bacc, mybir
from concourse.bass_utils import run_bass_kernel_spmd

F32 = mybir.dt.float32
F16 = mybir.dt.float16
AF = mybir.ActivationFunctionType

B, S, D = 4, 2048, 1024
HPC = 8          # heads per core
DHT = 512        # head dims per core (8 * 64)
NDT = D // 128   # 8 d-tiles (contraction tiles for projections)
NHT = DHT // 128  # 4 dh-tiles
NST = S // 128   # 16 s-tiles
NSB = S // 512   # 4 s-blocks
N_CORES = 8


def build_nc():
    nc = bacc.Bacc(None, target_bir_lowering=False)

    xq = nc.declare_dram_parameter("xq_t", [D, S], F16, isOutput=False)
    xk = nc.declare_dram_parameter("xk_t", [D, S], F16, isOutput=False)
    xv = nc.declare_dram_parameter("xv_t", [D, S], F16, isOutput=False)
    wq = nc.declare_dram_parameter("wq", [D, DHT], F16, isOutput=False)
    wk = nc.declare_dram_parameter("wk", [D, DHT], F16, isOutput=False)
    wv = nc.declare_dram_parameter("wv", [D, DHT], F16, isOutput=False)
    wo = nc.declare_dram_parameter("wo", [DHT, D], F16, isOutput=False)
    bq = nc.declare_dram_parameter("bq", [DHT], F32, isOutput=False)
    bk = nc.declare_dram_parameter("bk", [DHT], F32, isOutput=False)
    bv = nc.declare_dram_parameter("bv", [DHT], F32, isOutput=False)
    ot = nc.declare_dram_parameter("o_t", [D, S], F32, isOutput=True)

    # DRAM views tiled to 128 partitions
    xq_v = xq.rearrange("(t p) s -> p t s", p=128)
    xk_v = xk.rearrange("(t p) s -> p t s", p=128)
    xv_v = xv.rearrange("(t p) s -> p t s", p=128)
    wq_v = wq.rearrange("(t p) n -> p t n", p=128)
    wk_v = wk.rearrange("(t p) n -> p t n", p=128)
    wv_v = wv.rearrange("(t p) n -> p t n", p=128)
    wo_v = wo.rearrange("(t p) n -> p t n", p=128)
    ot_v = ot.rearrange("(t p) s -> t p s", p=128)

    with tile.TileContext(nc) as tc:
        with (
            tc.tile_pool(name="persist", bufs=1) as persist,
            tc.tile_pool(name="outp", bufs=4) as outp,
            tc.tile_pool(name="w3", bufs=1) as w3,
            tc.tile_pool(name="xs", bufs=4) as xs,
            tc.tile_pool(name="pexp_p", bufs=4) as pexp_p,
            tc.tile_pool(name="small", bufs=2) as small,
            tc.tile_pool(name="ps_big", bufs=2, space="PSUM") as ps_big,
            tc.tile_pool(name="ps_ctx", bufs=1, space="PSUM") as ps_ctx,
            tc.tile_pool(name="ps_o", bufs=2, space="PSUM") as ps_o,
        ):
            KT = persist.tile([128, NHT, S], F16)        # K^T  [dh, s]
            QT = persist.tile([128, NHT, S], F16)        # Q^T  [dh, s]
            Vt = persist.tile([128, NST, HPC, 65], F16)  # V natural + ones col
            ctxn = persist.tile([128, NHT, S], F16)      # normalized ctx^T
            wo_sb = persist.tile([128, NHT, D], F16)
            bq_sb = persist.tile([128, NHT], F32)
            bk_sb = persist.tile([128, NHT], F32)
            bv_bc = persist.tile([128, HPC, 64], F32)
            wq_sb = w3.tile([128, NDT, DHT], F16)
            wk_sb = w3.tile([128, NDT, DHT], F16)
            wv_sb = w3.tile([128, NDT, DHT], F16)

            nc.sync.dma_start(out=bq_sb, in_=bq.rearrange("(t p) -> p t", p=128))
            nc.sync.dma_start(out=bk_sb, in_=bk.rearrange("(t p) -> p t", p=128))
            nc.sync.dma_start(
                out=bv_bc,
                in_=bv.rearrange("(h d) -> h d", d=64).partition_broadcast(128),
            )
            # ones column for the PV matmul's softmax-denominator row
            nc.vector.memset(Vt[:, :, :, 64:65], 1.0)
            for dt in range(NDT):  # split DMAs across queues; K first
                nc.sync.dma_start(out=wk_sb[:, dt, :], in_=wk_v[:, dt, :])

            def emit_proj(kind, sb):
                ssl = slice(sb * 512, (sb + 1) * 512)
                xv_ap = {"k": xk_v, "v": xv_v, "q": xq_v}[kind]
                xst = xs.tile([128, NDT, 512], F16, tag="xs")
                for dt in range(NDT):
                    nc.sync.dma_start(out=xst[:, dt, :], in_=xv_ap[:, dt, ssl])
                if kind == "v":
                    # V projection, natural layout: the X^T tile is
                    # stationary so out[s-tile, dh] has s on partitions
                    for su in range(4):
                        pso = ps_o.tile([128, 512], F32, tag="po")
                        for dt in range(NDT):
                            nc.tensor.matmul(
                                pso[:, :],
                                xst[:, dt, bass.ts(su, 128)],
                                wv_sb[:, dt, :],
                                start=(dt == 0),
                                stop=(dt == NDT - 1),
                            )
                        nc.vector.tensor_add(
                            out=Vt[:, sb * 4 + su, :, 0:64],
                            in0=pso.rearrange("p (h d) -> p h d", d=64),
                            in1=bv_bc,
                        )
                else:
                    w_sb = wk_sb if kind == "k" else wq_sb
                    dst = KT if kind == "k" else QT
                    b_sb = bk_sb if kind == "k" else bq_sb
                    # K^T / Q^T: out[dh-tile, s-blk] = W^T-contract X^T
                    for ht in range(NHT):
                        ps = ps_o.tile([128, 512], F32, tag="po")
                        for dt in range(NDT):
                            nc.tensor.matmul(
                                ps[:, :],
                                w_sb[:, dt, bass.ts(ht, 128)],
                                xst[:, dt, :],
                                start=(dt == 0),
                                stop=(dt == NDT - 1),
                            )
                        # DVE (not ACT) so the scalar engine stays
                        # free for the softmax exps
                        nc.vector.tensor_scalar_add(
                            out=dst[:, ht, ssl],
                            in0=ps[:, :],
                            scalar1=b_sb[:, ht : ht + 1],
                        )

            def emit_attention(qp):
                q0 = qp * 1024
                for h in range(HPC):
                    po = 64 * (h % 2)
                    ht = h // 2
                    ctx = ps_ctx.tile([65, 1024], F32, tag="ctx")
                    for sk in range(NST):
                        st = ps_big.tile([128, 1024], F32, tag="big")
                        for j in range(2):
                            nc.tensor.matmul(
                                st[:, bass.ts(j, 512)],
                                KT[po : po + 64, ht, bass.ts(sk, 128)],
                                QT[po : po + 64, ht, q0 + j * 512 : q0 + (j + 1) * 512],
                                start=True,
                                stop=True,
                            )
                        pexp = pexp_p.tile([128, 1024], F16, tag="pexp")
                        nc.scalar.activation(
                            out=pexp, in_=st, func=AF.Exp, scale=0.125
                        )
                        for j in range(2):
                            nc.tensor.matmul(
                                ctx[:, bass.ts(j, 512)],
                                Vt[:, sk, h, :],
                                pexp[:, bass.ts(j, 512)],
                                start=(sk == 0),
                                stop=(sk == NST - 1),
                            )
                    # copy PSUM out fast to release the ctx bank for the
                    # next head; normalize from SBUF off the critical path.
                    # The sums row is DMA-reshaped across 128 partitions so
                    # the (8 cyc/elem) reciprocal runs 128-wide.
                    ctxc = small.tile([65, 1024], F32, tag="ctxc")
                    nc.vector.tensor_copy(out=ctxc, in_=ctx[:, :])
                    rr = small.tile([128, 8], F32, tag="rr")
                    nc.sync.dma_start(out=rr, in_=ctxc[64:65, :])
                    rrv = small.tile([128, 8], F32, tag="rrv")
                    nc.vector.reciprocal(out=rrv, in_=rr)
                    rinvrow = small.tile([1, 1024], F32, tag="rinvrow")
                    nc.sync.dma_start(out=rinvrow, in_=rrv)
                    rbc = small.tile([64, 1024], F32, tag="rbc")
                    nc.gpsimd.partition_broadcast(rbc, rinvrow)
                    nc.vector.tensor_mul(
                        out=ctxn[po : po + 64, ht, q0 : q0 + 1024],
                        in0=ctxc[0:64, :],
                        in1=rbc,
                    )
                # output projection for this q-pair
                for dot in range(8):
                    for j in range(2):
                        pso = ps_o.tile([128, 512], F32, tag="po")
                        for kt in range(NHT):
                            nc.tensor.matmul(
                                pso[:, :],
                                wo_sb[:, kt, bass.ts(dot, 128)],
                                ctxn[:, kt, q0 + j * 512 : q0 + (j + 1) * 512],
                                start=(kt == 0),
                                stop=(kt == NHT - 1),
                            )
                        osb = outp.tile([128, 512], F32, tag="osb")
                        nc.vector.tensor_copy(out=osb, in_=pso)
                        nc.sync.dma_start(
                            out=ot_v[dot, :, q0 + j * 512 : q0 + (j + 1) * 512],
                            in_=osb,
                        )

            # All projections are emitted first (program order = dependency
            # order), with the blocks attention(qp0) needs first. attention(
            # qp0) is then emitted at scheduler priority 0: it preempts the
            # deferred projections as soon as each of its inputs is placed,
            # and the remaining projection matmuls fill the PE slack of the
            # ACT(exp)-paced attention phase.
            emit_proj("k", 0)
            for dt in range(NDT):
                nc.sync.dma_start(out=wv_sb[:, dt, :], in_=wv_v[:, dt, :])
            emit_proj("v", 0)
            for dt in range(NDT):
                nc.sync.dma_start(out=wq_sb[:, dt, :], in_=wq_v[:, dt, :])
            emit_proj("q", 0)
            emit_proj("q", 1)
            for kt in range(NHT):
                nc.sync.dma_start(out=wo_sb[:, kt, :], in_=wo_v[:, kt, :])
            for sb in (1, 2, 3):
                emit_proj("k", sb)
                emit_proj("v", sb)
            emit_proj("q", 2)
            emit_proj("q", 3)
            with tc.high_priority():
                emit_attention(0)
            emit_attention(1)

    nc.compile()
    return nc


_NC_CACHE = None


def _get_nc():
    global _NC_CACHE
    if _NC_CACHE is None:
        _NC_CACHE = build_nc()
    return _NC_CACHE


def make_in_maps(q, k, v, Wq, bq, Wk, bk, Wv, bv, Wo):
    bf = np.float16
    in_maps = []
    for core in range(N_CORES):
        b, hg = core // 2, core % 2
        csl = slice(hg * DHT, (hg + 1) * DHT)
        in_maps.append(
            {
                "xq_t": np.ascontiguousarray(q[b].T).astype(bf),
                "xk_t": np.ascontiguousarray(k[b].T).astype(bf),
                "xv_t": np.ascontiguousarray(v[b].T).astype(bf),
                "wq": np.ascontiguousarray(Wq[:, csl]).astype(bf),
                "wk": np.ascontiguousarray(Wk[:, csl]).astype(bf),
                "wv": np.ascontiguousarray(Wv[:, csl]).astype(bf),
                "wo": np.ascontiguousarray(Wo[csl, :]).astype(bf),
                "bq": np.ascontiguousarray(bq[csl]).astype(np.float32),
                "bk": np.ascontiguousarray(bk[csl]).astype(np.float32),
                "bv": np.ascontiguousarray(bv[csl]).astype(np.float32),
            }
        )
    return in_maps


def kernel(q, k, v, Wq, bq, Wk, bk, Wv, bv, Wo, bo):
    q = np.asarray(q, np.float32)
    k = np.asarray(k, np.float32)
    v = np.asarray(v, np.float32)
    Wq = np.asarray(Wq, np.float32)
    Wk = np.asarray(Wk, np.float32)
    Wv = np.asarray(Wv, np.float32)
    Wo = np.asarray(Wo, np.float32)
    bq = np.asarray(bq, np.float32)
    bk = np.asarray(bk, np.float32)
    bv = np.asarray(bv, np.float32)
    bo = np.asarray(bo, np.float32)

    nc = _get_nc()
    in_maps = make_in_maps(q, k, v, Wq, bq, Wk, bk, Wv, bv, Wo)
    res = run_bass_kernel_spmd(nc, in_maps, list(range(N_CORES)))
    out = np.empty((B, S, D), np.float32)
    for b in range(B):
        o_t = res.results[2 * b]["o_t"] + res.results[2 * b + 1]["o_t"]
        out[b] = o_t.T + bo
    return out


# revision 19
# speedup vs baseline: 1.9514x; 1.0038x over previous
"""Multi-head attention (B=4, S=2048, D=1024, H=16) on 8 trn2 NeuronCores.

Sharding: (batch, head-group) -> 8 shards of (1 batch x 8 heads). Zero
cross-core communication: each core computes Q/K/V projections for its 8
heads, full attention over S=2048, and a partial output projection
(row-split Wo); the host sums the two head-group partials per batch.

Layout strategy: the host feeds per-batch inputs pre-transposed ([D, S]) so
every matmul's contraction dim lands on SBUF partitions without any
on-device transposes. The kernel keeps everything in "feature-major" form:
  Q^T, K^T: [dh, s]  -> scores^T[sk, q]  (softmax along partitions is
  avoided via an appended ones-column on V, which makes the PV matmul emit
  the softmax denominator as an extra output row)
  V: natural [s, dh] (+ ones col) -> ctx^T[dh(+1), q]
  out^T[do, q] = Wo_slice^T-contract(ctx^T / rowsum)

Matmul operands are fp16 (fp32 PSUM accumulation; the PE multiplies
bf16/fp16 at FP22 internally, so fp16's 10 mantissa bits survive). fp16
halves DMA volume and, unlike fp32/f32r, supports fast-weight-load +
background-buffer overlap, so the per-matmul LDWEIGHTS cost hides under
the previous matmul (f32r must self-load weights serially, measured ~2x
slower end-to-end). Simulated pipeline accuracy: ~7e-4 rel.
"""

import numpy as np

import concourse.bass as bass
import concourse.tile as tile
from concourse import bacc, mybir
from concourse.bass_utils import run_bass_kernel_spmd

F32 = mybir.dt.float32
F16 = mybir.dt.float16
AF = mybir.ActivationFunctionType

B, S, D = 4, 2048, 1024
HPC = 8          # heads per core
DHT = 512        # head dims per core (8 * 64)
NDT = D // 128   # 8 d-tiles (contraction tiles for projections)
NHT = DHT // 128  # 4 dh-tiles
NST = S // 128   # 16 s-tiles
NSB = S // 512   # 4 s-blocks
N_CORES = 8


def build_nc():
    nc = bacc.Bacc(None, target_bir_lowering=False)

    xq = nc.declare_dram_parameter("xq_t", [D, S], F16, isOutput=False)
    xk = nc.declare_dram_parameter("xk_t", [D, S], F16, isOutput=False)
    xv = nc.declare_dram_parameter("xv_t", [D, S], F16, isOutput=False)
    wq = nc.declare_dram_parameter("wq", [D, DHT], F16, isOutput=False)
    wk = nc.declare_dram_parameter("wk", [D, DHT], F16, isOutput=False)
    wv = nc.declare_dram_parameter("wv", [D, DHT], F16, isOutput=False)
    wo = nc.declare_dram_parameter("wo", [DHT, D], F16, isOutput=False)
    bq = nc.declare_dram_parameter("bq", [DHT], F32, isOutput=False)
    bk = nc.declare_dram_parameter("bk", [DHT], F32, isOutput=False)
    bv = nc.declare_dram_parameter("bv", [DHT], F32, isOutput=False)
    ot = nc.declare_dram_parameter("o_t", [D, S], F32, isOutput=True)

    # DRAM views tiled to 128 partitions
    xq_v = xq.rearrange("(t p) s -> p t s", p=128)
    xk_v = xk.rearrange("(t p) s -> p t s", p=128)
    xv_v = xv.rearrange("(t p) s -> p t s", p=128)
    wq_v = wq.rearrange("(t p) n -> p t n", p=128)
    wk_v = wk.rearrange("(t p) n -> p t n", p=128)
    wv_v = wv.rearrange("(t p) n -> p t n", p=128)
    wo_v = wo.rearrange("(t p) n -> p t n", p=128)
    ot_v = ot.rearrange("(t p) s -> t p s", p=128)

    with tile.TileContext(nc) as tc:
        with (
            tc.tile_pool(name="persist", bufs=1) as persist,
            tc.tile_pool(name="outp", bufs=4) as outp,
            tc.tile_pool(name="w3", bufs=1) as w3,
            tc.tile_pool(name="xs", bufs=4) as xs,
            tc.tile_pool(name="pexp_p", bufs=4) as pexp_p,
            tc.tile_pool(name="small", bufs=3) as small,
            tc.tile_pool(name="ps_big", bufs=2, space="PSUM") as ps_big,
            tc.tile_pool(name="ps_ctx", bufs=1, space="PSUM") as ps_ctx,
            tc.tile_pool(name="ps_o", bufs=2, space="PSUM") as ps_o,
        ):
            KT = persist.tile([128, NHT, S], F16)        # K^T  [dh, s]
            QT = persist.tile([128, NHT, S], F16)        # Q^T  [dh, s]
            Vt = persist.tile([128, NST, HPC, 65], F16)  # V natural + ones col
            ctxn = persist.tile([128, NHT, S], F16)      # normalized ctx^T
            wo_sb = persist.tile([128, NHT, D], F16)
            bq_sb = persist.tile([128, NHT], F32)
            bk_sb = persist.tile([128, NHT], F32)
            bv_bc = persist.tile([128, HPC, 64], F32)
            wq_sb = w3.tile([128, NDT, DHT], F16)
            wk_sb = w3.tile([128, NDT, DHT], F16)
            wv_sb = w3.tile([128, NDT, DHT], F16)

            nc.sync.dma_start(out=bq_sb, in_=bq.rearrange("(t p) -> p t", p=128))
            nc.sync.dma_start(out=bk_sb, in_=bk.rearrange("(t p) -> p t", p=128))
            nc.sync.dma_start(
                out=bv_bc,
                in_=bv.rearrange("(h d) -> h d", d=64).partition_broadcast(128),
            )
            # ones column for the PV matmul's softmax-denominator row
            nc.vector.memset(Vt[:, :, :, 64:65], 1.0)
            for dt in range(NDT):  # split DMAs across queues; K first
                nc.sync.dma_start(out=wk_sb[:, dt, :], in_=wk_v[:, dt, :])

            def emit_proj(kind, sb):
                ssl = slice(sb * 512, (sb + 1) * 512)
                xv_ap = {"k": xk_v, "v": xv_v, "q": xq_v}[kind]
                xst = xs.tile([128, NDT, 512], F16, tag="xs")
                for dt in range(NDT):
                    nc.sync.dma_start(out=xst[:, dt, :], in_=xv_ap[:, dt, ssl])
                if kind == "v":
                    # V projection, natural layout: the X^T tile is
                    # stationary so out[s-tile, dh] has s on partitions
                    for su in range(4):
                        pso = ps_o.tile([128, 512], F32, tag="po")
                        for dt in range(NDT):
                            nc.tensor.matmul(
                                pso[:, :],
                                xst[:, dt, bass.ts(su, 128)],
                                wv_sb[:, dt, :],
                                start=(dt == 0),
                                stop=(dt == NDT - 1),
                            )
                        nc.vector.tensor_add(
                            out=Vt[:, sb * 4 + su, :, 0:64],
                            in0=pso.rearrange("p (h d) -> p h d", d=64),
                            in1=bv_bc,
                        )
                else:
                    w_sb = wk_sb if kind == "k" else wq_sb
                    dst = KT if kind == "k" else QT
                    b_sb = bk_sb if kind == "k" else bq_sb
                    # K^T / Q^T: out[dh-tile, s-blk] = W^T-contract X^T
                    for ht in range(NHT):
                        ps = ps_o.tile([128, 512], F32, tag="po")
                        for dt in range(NDT):
                            nc.tensor.matmul(
                                ps[:, :],
                                w_sb[:, dt, bass.ts(ht, 128)],
                                xst[:, dt, :],
                                start=(dt == 0),
                                stop=(dt == NDT - 1),
                            )
                        # DVE (not ACT) so the scalar engine stays
                        # free for the softmax exps
                        nc.vector.tensor_scalar_add(
                            out=dst[:, ht, ssl],
                            in0=ps[:, :],
                            scalar1=b_sb[:, ht : ht + 1],
                        )

            def emit_attention(qp):
                q0 = qp * 1024
                for h in range(HPC):
                    po = 64 * (h % 2)
                    ht = h // 2
                    ctx = ps_ctx.tile([65, 1024], F32, tag="ctx")
                    for sk in range(NST):
                        st = ps_big.tile([128, 1024], F32, tag="big")
                        for j in range(2):
                            nc.tensor.matmul(
                                st[:, bass.ts(j, 512)],
                                KT[po : po + 64, ht, bass.ts(sk, 128)],
                                QT[po : po + 64, ht, q0 + j * 512 : q0 + (j + 1) * 512],
                                start=True,
                                stop=True,
                            )
                        pexp = pexp_p.tile([128, 1024], F16, tag="pexp")
                        nc.scalar.activation(
                            out=pexp, in_=st, func=AF.Exp, scale=0.125
                        )
                        for j in range(2):
                            nc.tensor.matmul(
                                ctx[:, bass.ts(j, 512)],
                                Vt[:, sk, h, :],
                                pexp[:, bass.ts(j, 512)],
                                start=(sk == 0),
                                stop=(sk == NST - 1),
                            )
                    # copy PSUM out fast to release the ctx bank for the
                    # next head; normalize from SBUF off the critical path.
                    # The sums row is DMA-reshaped across 128 partitions so
                    # the (8 cyc/elem) reciprocal runs 128-wide.
                    ctxc = small.tile([65, 1024], F32, tag="ctxc")
                    nc.vector.tensor_copy(out=ctxc, in_=ctx[:, :])
                    rr = small.tile([128, 8], F32, tag="rr")
                    nc.sync.dma_start(out=rr, in_=ctxc[64:65, :])
                    rrv = small.tile([128, 8], F32, tag="rrv")
                    nc.vector.reciprocal(out=rrv, in_=rr)
                    rinvrow = small.tile([1, 1024], F32, tag="rinvrow")
                    nc.sync.dma_start(out=rinvrow, in_=rrv)
                    rbc = small.tile([64, 1024], F32, tag="rbc")
                    nc.gpsimd.partition_broadcast(rbc, rinvrow)
                    nc.vector.tensor_mul(
                        out=ctxn[po : po + 64, ht, q0 : q0 + 1024],
                        in0=ctxc[0:64, :],
                        in1=rbc,
                    )
                # output projection for this q-pair
                for dot in range(8):
                    for j in range(2):
                        pso = ps_o.tile([128, 512], F32, tag="po")
                        for kt in range(NHT):
                            nc.tensor.matmul(
                                pso[:, :],
                                wo_sb[:, kt, bass.ts(dot, 128)],
                                ctxn[:, kt, q0 + j * 512 : q0 + (j + 1) * 512],
                                start=(kt == 0),
                                stop=(kt == NHT - 1),
                            )
                        osb = outp.tile([128, 512], F32, tag="osb")
                        nc.vector.tensor_copy(out=osb, in_=pso)
                        nc.sync.dma_start(
                            out=ot_v[dot, :, q0 + j * 512 : q0 + (j + 1) * 512],
                            in_=osb,
                        )

            # All projections are emitted first (program order = dependency
            # order), with the blocks attention(qp0) needs first. attention(
            # qp0) is then emitted at scheduler priority 0: it preempts the
            # deferred projections as soon as each of its inputs is placed,
            # and the remaining projection matmuls fill the PE slack of the
            # ACT(exp)-paced attention phase.
            emit_proj("k", 0)
            for dt in range(NDT):
                nc.sync.dma_start(out=wv_sb[:, dt, :], in_=wv_v[:, dt, :])
            emit_proj("v", 0)
            for dt in range(NDT):
                nc.sync.dma_start(out=wq_sb[:, dt, :], in_=wq_v[:, dt, :])
            emit_proj("q", 0)
            emit_proj("q", 1)
            for kt in range(NHT):
                nc.sync.dma_start(out=wo_sb[:, kt, :], in_=wo_v[:, kt, :])
            for sb in (1, 2, 3):
                emit_proj("k", sb)
                emit_proj("v", sb)
            emit_proj("q", 2)
            emit_proj("q", 3)
            with tc.high_priority():
                emit_attention(0)
            emit_attention(1)

    nc.compile()
    return nc


_NC_CACHE = None


def _get_nc():
    global _NC_CACHE
    if _NC_CACHE is None:
        _NC_CACHE = build_nc()
    return _NC_CACHE


def make_in_maps(q, k, v, Wq, bq, Wk, bk, Wv, bv, Wo):
    bf = np.float16
    in_maps = []
    for core in range(N_CORES):
        b, hg = core // 2, core % 2
        csl = slice(hg * DHT, (hg + 1) * DHT)
        in_maps.append(
            {
                "xq_t": np.ascontiguousarray(q[b].T).astype(bf),
                "xk_t": np.ascontiguousarray(k[b].T).astype(bf),
                "xv_t": np.ascontiguousarray(v[b].T).astype(bf),
                "wq": np.ascontiguousarray(Wq[:, csl]).astype(bf),
                "wk": np.ascontiguousarray(Wk[:, csl]).astype(bf),
                "wv": np.ascontiguousarray(Wv[:, csl]).astype(bf),
                "wo": np.ascontiguousarray(Wo[csl, :]).astype(bf),
                "bq": np.ascontiguousarray(bq[csl]).astype(np.float32),
                "bk": np.ascontiguousarray(bk[csl]).astype(np.float32),
                "bv": np.ascontiguousarray(bv[csl]).astype(np.float32),
            }
        )
    return in_maps


def kernel(q, k, v, Wq, bq, Wk, bk, Wv, bv, Wo, bo):
    q = np.asarray(q, np.float32)
    k = np.asarray(k, np.float32)
    v = np.asarray(v, np.float32)
    Wq = np.asarray(Wq, np.float32)
    Wk = np.asarray(Wk, np.float32)
    Wv = np.asarray(Wv, np.float32)
    Wo = np.asarray(Wo, np.float32)
    bq = np.asarray(bq, np.float32)
    bk = np.asarray(bk, np.float32)
    bv = np.asarray(bv, np.float32)
    bo = np.asarray(bo, np.float32)

    nc = _get_nc()
    in_maps = make_in_maps(q, k, v, Wq, bq, Wk, bk, Wv, bv, Wo)
    res = run_bass_kernel_spmd(nc, in_maps, list(range(N_CORES)))
    out = np.empty((B, S, D), np.float32)
    for b in range(B):
        o_t = res.results[2 * b]["o_t"] + res.results[2 * b + 1]["o_t"]
        out[b] = o_t.T + bo
    return out
